# revision 38
# baseline (speedup 1.0000x reference)
"""Trainium2 Bass kernel for multi-query attention with tanh-clamped softmax.

Sharding: tensor-parallel over the 8 query heads (one head per core, both
batches). K/V projections are small and replicated. The output projection is
row-parallel (each core computes a full-shape partial); the host sums the 8
partials (the unshard step for row-parallel tensor parallelism).

All matmuls run in bf16 with fp32 PSUM accumulation; layernorm statistics and
softmax normalization are computed in fp32.
"""

import os
import sys

sys.path.insert(0, "/opt/trn_rl_repo")

import numpy as np
import ml_dtypes

import concourse.bass as bass
import concourse.tile as tile
from concourse import bacc, mybir
from concourse.bass_utils import run_bass_kernel_spmd
from concourse.masks import make_identity

F32 = mybir.dt.float32
BF16 = mybir.dt.bfloat16
F8 = mybir.dt.float8e4
AF = mybir.ActivationFunctionType
ALU = mybir.AluOpType

HEADS = 8
DQK = 128
DV = 192
SCALE = 64 ** -0.5
CLAMP = 5.0
EPS = 1e-5

B = 2
N = 2048
DIM = 1536
N_CORES = 8

AV_FP8 = os.environ.get("AV_FP8", "1") == "1"

_LAST_STATS = {}


def build_nc(b=B, n=N, dim=DIM):
    """Build the per-core Bass graph. All cores run the same graph (SPMD)."""
    assert dim % 128 == 0 and n % 512 == 0
    DIMT = dim // 128          # dim tiles (contraction for projections)
    RT_PER_B = n // 128        # row tiles per batch
    KT = n // 128              # key tiles per batch
    QH = n // 1024 if n >= 1024 else 1
    QHW = n // QH              # qrows per qhalf pass
    QC = QHW // 512            # 512-wide q chunks per qhalf
    CC = dim // 512            # output column chunks
    WCOLS = DQK + DQK + DV     # 448

    nc = bacc.Bacc("TRN2", target_bir_lowering=False)

    xT = nc.declare_dram_parameter("xT", [dim, b * n], BF16, isOutput=False)
    w_all = nc.declare_dram_parameter("w_all", [dim, WCOLS], BF16, isOutput=False)
    biasT = nc.declare_dram_parameter("biasT", [b, n, n], BF16, isOutput=False)
    w_out = nc.declare_dram_parameter("w_out", [DV, dim], BF16, isOutput=False)
    gq = nc.declare_dram_parameter("gq", [DQK, 1], F32, isOutput=False)
    gk = nc.declare_dram_parameter("gk", [DQK, 1], F32, isOutput=False)
    out = nc.declare_dram_parameter("out", [b, n, dim], BF16, isOutput=True)

    with tile.TileContext(nc) as tc:
        with (
            tc.tile_pool(name="const", bufs=1) as const,
            tc.tile_pool(name="big", bufs=1) as big,
            tc.tile_pool(name="stA", bufs=5) as sA,
            tc.tile_pool(name="stB", bufs=3) as sB,
            tc.tile_pool(name="biasp", bufs=6) as sBias,
            tc.tile_pool(name="expp", bufs=4) as sE,
            tc.tile_pool(name="dramp", bufs=3, space="DRAM") as sDram,
            tc.tile_pool(name="work_ps", bufs=4, space="PSUM") as psW,
            tc.tile_pool(name="acc_ps", bufs=1, space="PSUM") as psAcc,
        ):
            # ---------------- constants ----------------
            # split along dt so the first projection matmuls can start after
            # half the weight/x data has landed
            DTH = DIMT // 2
            w_all_sb = const.tile([128, DIMT, WCOLS], BF16)
            w_all_r = w_all.rearrange("(t p) c -> p t c", p=128)
            nc.scalar.dma_start(out=w_all_sb[:, :DTH, :], in_=w_all_r[:, :DTH, :])
            nc.scalar.dma_start(out=w_all_sb[:, DTH:, :], in_=w_all_r[:, DTH:, :])
            gq_sb = const.tile([128, 1], F32)
            nc.sync.dma_start(out=gq_sb, in_=gq[:, :])
            gk_sb = const.tile([128, 1], F32)
            nc.sync.dma_start(out=gk_sb, in_=gk[:, :])
            ident = const.tile([128, 128], BF16)
            make_identity(nc, ident)
            eps_sb = const.tile([128, 1], F32)
            nc.vector.memset(eps_sb, EPS)

            # ---------------- resident activations ----------------
            NXC = 8                      # x chunks (columns of xT), streamed
            XCW = (b * n) // NXC
            xTr = xT.rearrange("(t p) r -> p t r", p=128)

            qT_sb = [big.tile([128, n], BF16, name=f"qT{bb}") for bb in range(b)]
            kT_sb = [big.tile([128, n], BF16, name=f"kT{bb}") for bb in range(b)]
            # kt-stride padded to 208 B (16B-aligned) for DoubleRow ldweights
            VDT = F8 if AV_FP8 else BF16
            v_sb = [big.tile([128, KT, 208], VDT, name=f"v{bb}") for bb in range(b)]
            for bb in range(b):
                nc.vector.memset(v_sb[bb][:, :, DV:DV + 1], 1.0)

            # ---------------- stage A: QKV projection + LN + transpose ----------------
            # Transposes are emitted with a 2-iteration skew so the PE queue
            # never blocks on the LN chain of the current row-tile. Gamma
            # (and SCALE for q) is applied on ScalarE during the PSUM->SBUF
            # copy of each transposed tile.
            RT_PER_XC = XCW // 128
            pending_tr = []

            def emit_tr(bb_, ktile_, qn_, kn_, on_dve=False):
                qtp = psW.tile([128, 512], BF16, name="qtp", tag="w")[:, :128]
                nc.tensor.transpose(qtp, qn_, ident)
                ktp = psW.tile([128, 512], BF16, name="ktp", tag="w")[:, :128]
                nc.tensor.transpose(ktp, kn_, ident)
                qdst = qT_sb[bb_][:, ktile_ * 128:(ktile_ + 1) * 128]
                kdst = kT_sb[bb_][:, ktile_ * 128:(ktile_ + 1) * 128]
                if on_dve:
                    # bf16 PSUM src -> 2x mode; keeps the evac off the busy
                    # ACT FIFO when pairs are injected into the attention loop
                    nc.vector.tensor_scalar_mul(out=qdst, in0=qtp, scalar1=gq_sb)
                    nc.vector.tensor_scalar_mul(out=kdst, in0=ktp, scalar1=gk_sb)
                else:
                    nc.scalar.activation(out=qdst, in_=qtp, func=AF.Copy,
                                         scale=gq_sb)
                    nc.scalar.activation(out=kdst, in_=ktp, func=AF.Copy,
                                         scale=gk_sb)

            _rt_state = {"mvp": None, "held": None}
            _xt_chunks = {}

            def load_xt(xc, engine=None):
                eng = engine or nc.sync
                xt_sb = sA.tile([128, DIMT, XCW], BF16, name="xt_sb", tag="xt")
                eng.dma_start(
                    out=xt_sb[:, :DTH, :],
                    in_=xTr[:, :DTH, xc * XCW:(xc + 1) * XCW],
                )
                eng.dma_start(
                    out=xt_sb[:, DTH:, :],
                    in_=xTr[:, DTH:, xc * XCW:(xc + 1) * XCW],
                )
                _xt_chunks[xc] = xt_sb

            def emit_rt(rt):
                bb = rt // RT_PER_B
                ktile = rt % RT_PER_B
                xc = rt // RT_PER_XC
                sub = rt % RT_PER_XC
                xoff = sub * 128
                if sub == 0 and xc not in _xt_chunks:
                    load_xt(xc)
                xt_sb = _xt_chunks[xc]

                qkv_ps = psW.tile([128, 512], F32, name="qkv_ps", tag="w")[:, :WCOLS]
                for dt_ in range(DIMT):
                    nc.tensor.matmul(
                        qkv_ps,
                        lhsT=xt_sb[:, dt_, xoff:xoff + 128],
                        rhs=w_all_sb[:, dt_, :],
                        start=(dt_ == 0),
                        stop=(dt_ == DIMT - 1),
                    )
                # single fast copy releases the PSUM slot; the LN chain (which
                # can sit behind the ACT/DVE backlog) reads the SBUF copy
                qkv_sb = sA.tile([128, WCOLS], F32, name="qkv_sb")
                nc.vector.tensor_copy(out=qkv_sb, in_=qkv_ps)
                if len(pending_tr) >= 2:
                    emit_tr(*pending_tr.pop(0))

                # layernorm stats for the three segments (q, k, v).
                # Stats for pairs of row-tiles share one mv tile so the
                # sqrt+reciprocal run once per pair (both have ~800ns fixed
                # cost); normalization is emitted on the odd row-tile.
                segs = [(0, DQK), (DQK, DQK), (2 * DQK, DV)]
                par = rt % 2
                if par == 0:
                    _rt_state["mvp"] = sA.tile([128, 2, 3, 2], F32, name="mvp")
                mvp = _rt_state["mvp"]
                stats = sA.tile([128, 3, 6], F32, name="stats")
                for si, (off, w) in enumerate(segs):
                    nc.vector.bn_stats(out=stats[:, si, :], in_=qkv_sb[:, off:off + w])
                    nc.vector.bn_aggr(out=mvp[:, par, si, :], in_=stats[:, si, :])
                if par == 0:
                    _rt_state["held"] = (bb, ktile, qkv_sb)
                    return
                held = _rt_state["held"]
                rstd = sA.tile([128, 2, 3], F32, name="rstd")
                nc.scalar.activation(rstd, mvp[:, :, :, 1], AF.Sqrt, bias=eps_sb)
                nc.vector.reciprocal(out=rstd, in_=rstd)

                for pp, (bb_, ktile_, ps_) in enumerate([held, (bb, ktile, qkv_sb)]):
                    qn = sA.tile([128, 128], BF16, name="qn")
                    nc.vector.tensor_scalar(
                        out=qn, in0=ps_[:, 0:DQK],
                        scalar1=mvp[:, pp, 0, 0:1], scalar2=rstd[:, pp, 0:1],
                        op0=ALU.subtract, op1=ALU.mult,
                    )
                    kn = sA.tile([128, 128], BF16, name="kn")
                    nc.vector.tensor_scalar(
                        out=kn, in0=ps_[:, DQK:2 * DQK],
                        scalar1=mvp[:, pp, 1, 0:1], scalar2=rstd[:, pp, 1:2],
                        op0=ALU.subtract, op1=ALU.mult,
                    )
                    nc.vector.tensor_scalar(
                        out=v_sb[bb_][:, ktile_, 0:DV], in0=ps_[:, 2 * DQK:WCOLS],
                        scalar1=mvp[:, pp, 2, 0:1], scalar2=rstd[:, pp, 2:3],
                        op0=ALU.subtract, op1=ALU.mult,
                    )
                    pending_tr.append((bb_, ktile_, qn, kn, bb_ == 1))

            def flush_tr():
                while pending_tr:
                    emit_tr(*pending_tr.pop(0))

            # batch 0's projection runs standalone; batch 1's row-tile pairs
            # are injected into batch 0's attention loop below so the PE
            # queue stays dense across the phase boundary.
            for rt in range(RT_PER_B):
                emit_rt(rt)
            flush_tr()
            # prefetch batch 1's x chunks on the (idle) scalar HWDGE queue so
            # the injected projection matmuls never block the PE FIFO on DMA
            for xc in range(RT_PER_B // RT_PER_XC, (b * RT_PER_B) // RT_PER_XC):
                load_xt(xc, engine=nc.scalar)
            inject_pairs = list(range(RT_PER_B, b * RT_PER_B, 2))

            # ---------------- stage B: attention + output projection ----------------
            # attn@v matmuls are emitted one kt-iteration behind their sim so
            # the PE never blocks on the DVE->ACT->ACT chain; the previous
            # qhalf's output projection is drip-fed into the kt loop.
            # w_out is loaded here (not with the other constants) so the head
            # DMA queue serves stage A's x/w tiles first.
            w_out_a = const.tile([128, dim], BF16)
            nc.sync.dma_start(out=w_out_a, in_=w_out[0:128, :])
            w_out_b = const.tile([64, dim], BF16)
            nc.sync.dma_start(out=w_out_b, in_=w_out[128:192, :])

            def emit_po(outUa_, outUb_, rcol_, bb_, qoff_, t, cc, tail=False):
                po = psW.tile([128, 512], F32, name="po", tag="w")
                nc.tensor.matmul(
                    po,
                    lhsT=outUa_[:, t * 128:(t + 1) * 128],
                    rhs=w_out_a[:, cc * 512:(cc + 1) * 512],
                    start=True, stop=False,
                )
                nc.tensor.matmul(
                    po,
                    lhsT=outUb_[:, t * 128:(t + 1) * 128],
                    rhs=w_out_b[:, cc * 512:(cc + 1) * 512],
                    start=False, stop=True,
                )
                po_sb = sB.tile([128, 512], BF16, name="po_sb")
                on_act = (t * CC + cc) % 2 == 1 if tail else (t * CC + cc) % 3 == 2
                if on_act:
                    nc.scalar.activation(
                        out=po_sb, in_=po, func=AF.Copy, scale=rcol_[:, t:t + 1])
                else:
                    nc.vector.tensor_scalar_mul(
                        out=po_sb, in0=po, scalar1=rcol_[:, t:t + 1])
                nc.sync.dma_start(
                    out=out[bb_, qoff_ + t * 128: qoff_ + (t + 1) * 128,
                            cc * 512:(cc + 1) * 512],
                    in_=po_sb,
                )

            import functools

            KT2 = KT // 2
            pending_po = []
            pending_evac = []
            for bb in range(b):
                for qh in range(QH):
                    qoff = qh * QHW
                    accA = [psAcc.tile([128, 512], F32, name=f"accA{qc}") for qc in range(QC)]
                    accB = [psAcc.tile([65, 512], F32, name=f"accB{qc}") for qc in range(QC)]

                    def fire_av(p, ep):
                        if AV_FP8:
                            # fp8 DoubleRow: one matmul covers key-tiles 2p, 2p+1.
                            for qc in range(QC):
                                nc.tensor.matmul(
                                    accA[qc],
                                    lhsT=v_sb[bb][:, 2 * p:2 * p + 2, 0:128],
                                    rhs=ep[:, :, qc * 512:(qc + 1) * 512],
                                    start=(p == 0), stop=(p == KT2 - 1),
                                    perf_mode=mybir.MatmulPerfMode.DoubleRow,
                                )
                                nc.tensor.matmul(
                                    accB[qc],
                                    lhsT=v_sb[bb][:, 2 * p:2 * p + 2, 128:DV + 1],
                                    rhs=ep[:, :, qc * 512:(qc + 1) * 512],
                                    start=(p == 0), stop=(p == KT2 - 1),
                                    perf_mode=mybir.MatmulPerfMode.DoubleRow,
                                )
                        else:
                            for s in range(2):
                                pkt = 2 * p + s
                                for qc in range(QC):
                                    nc.tensor.matmul(
                                        accA[qc],
                                        lhsT=v_sb[bb][:, pkt, 0:128],
                                        rhs=ep[:, s, qc * 512:(qc + 1) * 512],
                                        start=(pkt == 0), stop=(pkt == KT - 1),
                                    )
                                    nc.tensor.matmul(
                                        accB[qc],
                                        lhsT=v_sb[bb][:, pkt, 128:DV + 1],
                                        rhs=ep[:, s, qc * 512:(qc + 1) * 512],
                                        start=(pkt == 0), stop=(pkt == KT - 1),
                                    )

                    pending_av = []
                    e_pair = None
                    for kt in range(KT):
                        th_sb = sB.tile([128, QHW], F32, name="th_sb")
                        for qc in range(QC):
                            bias_sb = sBias.tile([128, 512], BF16, name="bias_sb")
                            nc.sync.dma_start(
                                out=bias_sb,
                                in_=biasT[bb, kt * 128:(kt + 1) * 128,
                                          qoff + qc * 512: qoff + (qc + 1) * 512],
                            )
                            sim_ps = psW.tile([128, 512], F32, name="sim_ps", tag="w")
                            nc.tensor.matmul(
                                sim_ps,
                                lhsT=kT_sb[bb][:, kt * 128:(kt + 1) * 128],
                                rhs=qT_sb[bb][:, qoff + qc * 512: qoff + (qc + 1) * 512],
                                start=True, stop=False,
                            )
                            # bias-add on the PE: identity matmul accumulates
                            # the bias tile into the sim PSUM bank, keeping
                            # the sim->tanh chain off the (FIFO-ordered) DVE.
                            nc.tensor.matmul(
                                sim_ps,
                                lhsT=ident,
                                rhs=bias_sb,
                                start=False, stop=True,
                            )
                            nc.scalar.activation(
                                th_sb[:, qc * 512:(qc + 1) * 512], sim_ps,
                                AF.Tanh, scale=1.0 / CLAMP)
                        if kt % 2 == 0:
                            e_pair = sE.tile([128, 2, QHW], VDT, name="e_pair")
                        nc.scalar.activation(e_pair[:, kt % 2, :], th_sb,
                                             AF.Exp, scale=CLAMP)
                        if kt % 2 == 1:
                            pending_av.append((kt // 2, e_pair))

                        if len(pending_av) >= 3:
                            fire_av(*pending_av.pop(0))

                        if bb == 0 and kt % 4 == 2 and inject_pairs:
                            r0 = inject_pairs.pop(0)
                            emit_rt(r0)
                            emit_rt(r0 + 1)

                        if kt >= 2:
                            for _ in range(2):
                                if pending_po:
                                    pending_po.pop(0)()

                    for p, ep in pending_av:
                        fire_av(p, ep)

                    def make_evac(accA_, accB_, bb_, qoff_):  # noqa: unused-now-eager
                        # Evacuate accumulators with plain casts. The
                        # attention row-sums (65th accB row) are reshaped into
                        # per-partition columns via a DRAM bounce, then one
                        # wide reciprocal gives the 1/s scale column that the
                        # output-projection copy applies per token.
                        def evac():
                            outUa = sB.tile([128, QHW], BF16, name="outUa")
                            outUb = sB.tile([64, QHW], BF16, name="outUb")
                            s_row = sB.tile([1, QHW], F32, name="s_row")
                            for qc in range(QC):
                                nc.vector.tensor_copy(
                                    out=s_row[:, qc * 512:(qc + 1) * 512],
                                    in_=accB_[qc][64:65, :],
                                )
                            s_dram = sDram.tile([1, QHW], F32, name="s_dram")
                            nc.sync.dma_start(out=s_dram, in_=s_row)
                            for qc in range(QC):
                                nc.vector.tensor_copy(
                                    out=outUa[:, qc * 512:(qc + 1) * 512],
                                    in_=accA_[qc])
                                nc.vector.tensor_copy(
                                    out=outUb[:, qc * 512:(qc + 1) * 512],
                                    in_=accB_[qc][0:64, :])
                            rcol_raw = sB.tile([128, QHW // 128], F32,
                                               name="rcol_raw")
                            nc.sync.dma_start(
                                out=rcol_raw,
                                in_=s_dram.rearrange("one (t p) -> p (one t)",
                                                     p=128),
                            )
                            rcol = sB.tile([128, QHW // 128], F32, name="rcol")
                            nc.vector.reciprocal(out=rcol, in_=rcol_raw)
                            for t in range(QHW // 128):
                                for cc in range(CC):
                                    pending_po.append(functools.partial(
                                        emit_po, outUa, outUb, rcol,
                                        bb_, qoff_, t, cc))
                        return evac

                    make_evac(accA, accB, bb, qoff)()

                if bb == 0:
                    flush_tr()

            for fn in pending_po:
                fn(tail=True)

    nc.compile()
    return nc


_NC_CACHE = {}


def _get_nc(b=B, n=N, dim=DIM):
    key = (b, n, dim)
    if key not in _NC_CACHE:
        _NC_CACHE[key] = build_nc(b, n, dim)
    return _NC_CACHE[key]


def make_in_maps(x, attn_bias, w_qkv, w_out, g_q, g_k, g_v, n_cores=N_CORES):
    """Host-side shard + preprocess. Returns per-core input maps."""
    b, n, dim = x.shape
    bf = ml_dtypes.bfloat16
    xT = np.ascontiguousarray(
        x.reshape(b * n, dim).T).astype(bf)                      # [dim, b*n]
    kv_cols = np.ascontiguousarray(
        w_qkv[:, HEADS * DQK:]).astype(np.float32)               # [dim, 320]
    in_maps = []
    for c in range(n_cores):
        h = c % HEADS
        w_q_h = w_qkv[:, h * DQK:(h + 1) * DQK]
        w_all = np.concatenate([w_q_h, kv_cols], axis=1).astype(bf)  # [dim, 448]
        biasT = np.ascontiguousarray(
            attn_bias[:, h, :, :].transpose(0, 2, 1)).astype(bf)  # [b, keys, qrows]
        w_out_h = (w_out[h * DV:(h + 1) * DV, :]
                   * g_v[:, None].astype(np.float32)).astype(bf)  # [dv, dim]
        in_maps.append({
            "xT": xT,
            "w_all": w_all,
            "biasT": biasT,
            "w_out": w_out_h,
            "gq": (g_q * SCALE).astype(np.float32).reshape(DQK, 1),
            "gk": g_k.astype(np.float32).reshape(DQK, 1),
        })
    return in_maps


def kernel(x, attn_bias, w_qkv, w_out, g_q, g_k, g_v):
    x = np.asarray(x, dtype=np.float32)
    attn_bias = np.asarray(attn_bias, dtype=np.float32)
    w_qkv = np.asarray(w_qkv, dtype=np.float32)
    w_out = np.asarray(w_out, dtype=np.float32)
    g_q = np.asarray(g_q, dtype=np.float32)
    g_k = np.asarray(g_k, dtype=np.float32)
    g_v = np.asarray(g_v, dtype=np.float32)

    b, n, dim = x.shape
    nc = _get_nc(b, n, dim)
    in_maps = make_in_maps(x, attn_bias, w_qkv, w_out, g_q, g_k, g_v)
    res = run_bass_kernel_spmd(nc, in_maps, core_ids=list(range(N_CORES)),
                               trace=bool(os.environ.get("KERNEL_TRACE")))
    _LAST_STATS["exec_time_ns"] = res.exec_time_ns
    _LAST_STATS["mean_exec_time_ns"] = res.mean_exec_time_ns
    _LAST_STATS["res"] = res
    out = np.zeros((b, n, dim), dtype=np.float32)
    for c in range(N_CORES):
        out += res.results[c]["out"].astype(np.float32)
    return out



# revision 39
# speedup vs baseline: 1.1109x; 1.1109x over previous
"""Trainium2 Bass kernel for multi-query attention with tanh-clamped softmax.

Sharding: tensor-parallel over the 8 query heads (one head per core, both
batches). K/V projections are small and replicated. The output projection is
row-parallel (each core computes a full-shape partial); the host sums the 8
partials (the unshard step for row-parallel tensor parallelism).

All matmuls run in bf16 with fp32 PSUM accumulation; layernorm statistics and
softmax normalization are computed in fp32.
"""

import os
import sys

sys.path.insert(0, "/opt/trn_rl_repo")

import numpy as np
import ml_dtypes

import concourse.bass as bass
import concourse.tile as tile
from concourse import bacc, mybir
from concourse.bass_utils import run_bass_kernel_spmd
from concourse.masks import make_identity

F32 = mybir.dt.float32
BF16 = mybir.dt.bfloat16
F8 = mybir.dt.float8e4
AF = mybir.ActivationFunctionType
ALU = mybir.AluOpType

HEADS = 8
DQK = 128
DV = 192
SCALE = 64 ** -0.5
CLAMP = 5.0
EPS = 1e-5

B = 2
N = 2048
DIM = 1536
N_CORES = 8

AV_FP8 = os.environ.get("AV_FP8", "1") == "1"

_LAST_STATS = {}


def build_nc(b=B, n=N, dim=DIM):
    """Build the per-core Bass graph. All cores run the same graph (SPMD)."""
    assert dim % 128 == 0 and n % 512 == 0
    DIMT = dim // 128          # dim tiles (contraction for projections)
    RT_PER_B = n // 128        # row tiles per batch
    KT = n // 128              # key tiles per batch
    QH = n // 1024 if n >= 1024 else 1
    QHW = n // QH              # qrows per qhalf pass
    QC = QHW // 512            # 512-wide q chunks per qhalf
    CC = dim // 512            # output column chunks
    WCOLS = DQK + DQK + DV     # 448

    nc = bacc.Bacc("TRN2", target_bir_lowering=False)

    xT = nc.declare_dram_parameter("xT", [dim, b * n], BF16, isOutput=False)
    w_all = nc.declare_dram_parameter("w_all", [dim, WCOLS], BF16, isOutput=False)
    biasT = nc.declare_dram_parameter("biasT", [b, n, n], BF16, isOutput=False)
    w_out = nc.declare_dram_parameter("w_out", [DV, dim], BF16, isOutput=False)
    gq = nc.declare_dram_parameter("gq", [DQK, 1], F32, isOutput=False)
    gk = nc.declare_dram_parameter("gk", [DQK, 1], F32, isOutput=False)
    out = nc.declare_dram_parameter("out", [b, n, dim], BF16, isOutput=True)

    with tile.TileContext(nc) as tc:
        with (
            tc.tile_pool(name="const", bufs=1) as const,
            tc.tile_pool(name="big", bufs=1) as big,
            tc.tile_pool(name="stA", bufs=5) as sA,
            tc.tile_pool(name="stB", bufs=3) as sB,
            tc.tile_pool(name="biasp", bufs=6) as sBias,
            tc.tile_pool(name="expp", bufs=4) as sE,
            tc.tile_pool(name="dramp", bufs=3, space="DRAM") as sDram,
            tc.tile_pool(name="work_ps", bufs=4, space="PSUM") as psW,
            tc.tile_pool(name="acc_ps", bufs=1, space="PSUM") as psAcc,
        ):
            # ---------------- constants ----------------
            # split along dt so the first projection matmuls can start after
            # half the weight/x data has landed
            DTH = DIMT // 2
            w_all_sb = const.tile([128, DIMT, WCOLS], BF16)
            w_all_r = w_all.rearrange("(t p) c -> p t c", p=128)
            nc.scalar.dma_start(out=w_all_sb[:, :DTH, :], in_=w_all_r[:, :DTH, :])
            nc.scalar.dma_start(out=w_all_sb[:, DTH:, :], in_=w_all_r[:, DTH:, :])
            gq_sb = const.tile([128, 1], F32)
            nc.sync.dma_start(out=gq_sb, in_=gq[:, :])
            gk_sb = const.tile([128, 1], F32)
            nc.sync.dma_start(out=gk_sb, in_=gk[:, :])
            ident = const.tile([128, 128], BF16)
            make_identity(nc, ident)
            eps_sb = const.tile([128, 1], F32)
            nc.vector.memset(eps_sb, EPS)

            # ---------------- resident activations ----------------
            NXC = 8                      # x chunks (columns of xT), streamed
            XCW = (b * n) // NXC
            xTr = xT.rearrange("(t p) r -> p t r", p=128)

            qT_sb = [big.tile([128, n], BF16, name=f"qT{bb}") for bb in range(b)]
            kT_sb = [big.tile([128, n], BF16, name=f"kT{bb}") for bb in range(b)]
            # kt-stride padded to 208 B (16B-aligned) for DoubleRow ldweights
            VDT = F8 if AV_FP8 else BF16
            v_sb = [big.tile([128, KT, 208], VDT, name=f"v{bb}") for bb in range(b)]
            for bb in range(b):
                nc.vector.memset(v_sb[bb][:, :, DV:DV + 1], 1.0)

            # ---------------- stage A: QKV projection + LN + transpose ----------------
            # Transposes are emitted with a 2-iteration skew so the PE queue
            # never blocks on the LN chain of the current row-tile. Gamma
            # (and SCALE for q) is applied on ScalarE during the PSUM->SBUF
            # copy of each transposed tile.
            RT_PER_XC = XCW // 128
            pending_tr = []

            def emit_tr(bb_, ktile_, qn_, kn_, on_dve=False):
                qtp = psW.tile([128, 512], BF16, name="qtp", tag="w")[:, :128]
                nc.tensor.transpose(qtp, qn_, ident)
                ktp = psW.tile([128, 512], BF16, name="ktp", tag="w")[:, :128]
                nc.tensor.transpose(ktp, kn_, ident)
                qdst = qT_sb[bb_][:, ktile_ * 128:(ktile_ + 1) * 128]
                kdst = kT_sb[bb_][:, ktile_ * 128:(ktile_ + 1) * 128]
                if on_dve:
                    # bf16 PSUM src -> 2x mode; keeps the evac off the busy
                    # ACT FIFO when pairs are injected into the attention loop
                    nc.vector.tensor_scalar_mul(out=qdst, in0=qtp, scalar1=gq_sb)
                    nc.vector.tensor_scalar_mul(out=kdst, in0=ktp, scalar1=gk_sb)
                else:
                    nc.scalar.activation(out=qdst, in_=qtp, func=AF.Copy,
                                         scale=gq_sb)
                    nc.scalar.activation(out=kdst, in_=ktp, func=AF.Copy,
                                         scale=gk_sb)

            _rt_state = {"mvp": None, "held": None}
            _xt_chunks = {}

            def load_xt(xc, engine=None):
                eng = engine or nc.sync
                xt_sb = sA.tile([128, DIMT, XCW], BF16, name="xt_sb", tag="xt")
                eng.dma_start(
                    out=xt_sb[:, :DTH, :],
                    in_=xTr[:, :DTH, xc * XCW:(xc + 1) * XCW],
                )
                eng.dma_start(
                    out=xt_sb[:, DTH:, :],
                    in_=xTr[:, DTH:, xc * XCW:(xc + 1) * XCW],
                )
                _xt_chunks[xc] = xt_sb

            def emit_rt(rt):
                bb = rt // RT_PER_B
                ktile = rt % RT_PER_B
                xc = rt // RT_PER_XC
                sub = rt % RT_PER_XC
                xoff = sub * 128
                if sub == 0 and xc not in _xt_chunks:
                    load_xt(xc)
                xt_sb = _xt_chunks[xc]

                qkv_ps = psW.tile([128, 512], F32, name="qkv_ps", tag="w")[:, :WCOLS]
                for dt_ in range(DIMT):
                    nc.tensor.matmul(
                        qkv_ps,
                        lhsT=xt_sb[:, dt_, xoff:xoff + 128],
                        rhs=w_all_sb[:, dt_, :],
                        start=(dt_ == 0),
                        stop=(dt_ == DIMT - 1),
                    )
                # single fast copy releases the PSUM slot; the LN chain (which
                # can sit behind the ACT/DVE backlog) reads the SBUF copy
                qkv_sb = sA.tile([128, WCOLS], F32, name="qkv_sb")
                nc.vector.tensor_copy(out=qkv_sb, in_=qkv_ps)
                if len(pending_tr) >= 2:
                    emit_tr(*pending_tr.pop(0))

                # layernorm stats for the three segments (q, k, v).
                # Stats for pairs of row-tiles share one mv tile so the
                # sqrt+reciprocal run once per pair (both have ~800ns fixed
                # cost); normalization is emitted on the odd row-tile.
                segs = [(0, DQK), (DQK, DQK), (2 * DQK, DV)]
                par = rt % 2
                if par == 0:
                    _rt_state["mvp"] = sA.tile([128, 2, 3, 2], F32, name="mvp")
                mvp = _rt_state["mvp"]
                stats = sA.tile([128, 3, 6], F32, name="stats")
                for si, (off, w) in enumerate(segs):
                    nc.vector.bn_stats(out=stats[:, si, :], in_=qkv_sb[:, off:off + w])
                    nc.vector.bn_aggr(out=mvp[:, par, si, :], in_=stats[:, si, :])
                if par == 0:
                    _rt_state["held"] = (bb, ktile, qkv_sb)
                    return
                held = _rt_state["held"]
                rstd = sA.tile([128, 2, 3], F32, name="rstd")
                nc.scalar.activation(rstd, mvp[:, :, :, 1], AF.Sqrt, bias=eps_sb)
                nc.vector.reciprocal(out=rstd, in_=rstd)

                for pp, (bb_, ktile_, ps_) in enumerate([held, (bb, ktile, qkv_sb)]):
                    qn = sA.tile([128, 128], BF16, name="qn")
                    nc.vector.tensor_scalar(
                        out=qn, in0=ps_[:, 0:DQK],
                        scalar1=mvp[:, pp, 0, 0:1], scalar2=rstd[:, pp, 0:1],
                        op0=ALU.subtract, op1=ALU.mult,
                    )
                    kn = sA.tile([128, 128], BF16, name="kn")
                    nc.vector.tensor_scalar(
                        out=kn, in0=ps_[:, DQK:2 * DQK],
                        scalar1=mvp[:, pp, 1, 0:1], scalar2=rstd[:, pp, 1:2],
                        op0=ALU.subtract, op1=ALU.mult,
                    )
                    nc.vector.tensor_scalar(
                        out=v_sb[bb_][:, ktile_, 0:DV], in0=ps_[:, 2 * DQK:WCOLS],
                        scalar1=mvp[:, pp, 2, 0:1], scalar2=rstd[:, pp, 2:3],
                        op0=ALU.subtract, op1=ALU.mult,
                    )
                    pending_tr.append((bb_, ktile_, qn, kn, bb_ == 1))

            def flush_tr():
                while pending_tr:
                    emit_tr(*pending_tr.pop(0))

            # batch 0's projection runs standalone; batch 1's row-tile pairs
            # are injected into batch 0's attention loop below so the PE
            # queue stays dense across the phase boundary.
            for rt in range(RT_PER_B):
                emit_rt(rt)
            flush_tr()
            # prefetch batch 1's x chunks on the (idle) scalar HWDGE queue so
            # the injected projection matmuls never block the PE FIFO on DMA
            for xc in range(RT_PER_B // RT_PER_XC, (b * RT_PER_B) // RT_PER_XC):
                load_xt(xc, engine=nc.scalar)
            inject_pairs = list(range(RT_PER_B, b * RT_PER_B, 2))

            # ---------------- stage B: attention + output projection ----------------
            # attn@v matmuls are emitted one kt-iteration behind their sim so
            # the PE never blocks on the DVE->ACT->ACT chain; the previous
            # qhalf's output projection is drip-fed into the kt loop.
            # w_out is loaded here (not with the other constants) so the head
            # DMA queue serves stage A's x/w tiles first.
            w_out_a = const.tile([128, dim], BF16)
            nc.sync.dma_start(out=w_out_a, in_=w_out[0:128, :])
            w_out_b = const.tile([64, dim], BF16)
            nc.sync.dma_start(out=w_out_b, in_=w_out[128:192, :])

            def emit_po(outUa_, outUb_, rcol_, bb_, qoff_, t, cc, tail=False):
                po = psW.tile([128, 512], F32, name="po", tag="w")
                nc.tensor.matmul(
                    po,
                    lhsT=outUa_[:, t * 128:(t + 1) * 128],
                    rhs=w_out_a[:, cc * 512:(cc + 1) * 512],
                    start=True, stop=False,
                )
                nc.tensor.matmul(
                    po,
                    lhsT=outUb_[:, t * 128:(t + 1) * 128],
                    rhs=w_out_b[:, cc * 512:(cc + 1) * 512],
                    start=False, stop=True,
                )
                po_sb = sB.tile([128, 512], BF16, name="po_sb")
                on_act = (t * CC + cc) % 2 == 1 if tail else (t * CC + cc) % 3 == 2
                if on_act:
                    nc.scalar.activation(
                        out=po_sb, in_=po, func=AF.Copy, scale=rcol_[:, t:t + 1])
                else:
                    nc.vector.tensor_scalar_mul(
                        out=po_sb, in0=po, scalar1=rcol_[:, t:t + 1])
                nc.sync.dma_start(
                    out=out[bb_, qoff_ + t * 128: qoff_ + (t + 1) * 128,
                            cc * 512:(cc + 1) * 512],
                    in_=po_sb,
                )

            import functools

            KT2 = KT // 2
            pending_po = []
            pending_evac = []
            for bb in range(b):
                for qh in range(QH):
                    qoff = qh * QHW
                    accA = [psAcc.tile([128, 512], F32, name=f"accA{qc}") for qc in range(QC)]
                    accB = [psAcc.tile([65, 512], F32, name=f"accB{qc}") for qc in range(QC)]

                    def fire_av(p, ep):
                        if AV_FP8:
                            # fp8 DoubleRow: one matmul covers key-tiles 2p, 2p+1.
                            for qc in range(QC):
                                nc.tensor.matmul(
                                    accA[qc],
                                    lhsT=v_sb[bb][:, 2 * p:2 * p + 2, 0:128],
                                    rhs=ep[:, :, qc * 512:(qc + 1) * 512],
                                    start=(p == 0), stop=(p == KT2 - 1),
                                    perf_mode=mybir.MatmulPerfMode.DoubleRow,
                                )
                                nc.tensor.matmul(
                                    accB[qc],
                                    lhsT=v_sb[bb][:, 2 * p:2 * p + 2, 128:DV + 1],
                                    rhs=ep[:, :, qc * 512:(qc + 1) * 512],
                                    start=(p == 0), stop=(p == KT2 - 1),
                                    perf_mode=mybir.MatmulPerfMode.DoubleRow,
                                )
                        else:
                            for s in range(2):
                                pkt = 2 * p + s
                                for qc in range(QC):
                                    nc.tensor.matmul(
                                        accA[qc],
                                        lhsT=v_sb[bb][:, pkt, 0:128],
                                        rhs=ep[:, s, qc * 512:(qc + 1) * 512],
                                        start=(pkt == 0), stop=(pkt == KT - 1),
                                    )
                                    nc.tensor.matmul(
                                        accB[qc],
                                        lhsT=v_sb[bb][:, pkt, 128:DV + 1],
                                        rhs=ep[:, s, qc * 512:(qc + 1) * 512],
                                        start=(pkt == 0), stop=(pkt == KT - 1),
                                    )

                    pending_av = []
                    e_pair = None
                    for kt in range(KT):
                        th_sb = sB.tile([128, QHW], F32, name="th_sb")
                        for qc in range(QC):
                            bias_sb = sBias.tile([128, 512], BF16, name="bias_sb")
                            nc.sync.dma_start(
                                out=bias_sb,
                                in_=biasT[bb, kt * 128:(kt + 1) * 128,
                                          qoff + qc * 512: qoff + (qc + 1) * 512],
                            )
                            sim_ps = psW.tile([128, 512], F32, name="sim_ps", tag="w")
                            nc.tensor.matmul(
                                sim_ps,
                                lhsT=kT_sb[bb][:, kt * 128:(kt + 1) * 128],
                                rhs=qT_sb[bb][:, qoff + qc * 512: qoff + (qc + 1) * 512],
                                start=True, stop=False,
                            )
                            # bias-add on the PE: identity matmul accumulates
                            # the bias tile into the sim PSUM bank, keeping
                            # the sim->tanh chain off the (FIFO-ordered) DVE.
                            nc.tensor.matmul(
                                sim_ps,
                                lhsT=ident,
                                rhs=bias_sb,
                                start=False, stop=True,
                            )
                            nc.scalar.activation(
                                th_sb[:, qc * 512:(qc + 1) * 512], sim_ps,
                                AF.Tanh, scale=1.0 / CLAMP)
                        if kt % 2 == 0:
                            e_pair = sE.tile([128, 2, QHW], VDT, name="e_pair")
                        nc.scalar.activation(e_pair[:, kt % 2, :], th_sb,
                                             AF.Exp, scale=CLAMP)
                        if kt % 2 == 1:
                            pending_av.append((kt // 2, e_pair))

                        if len(pending_av) >= 3:
                            fire_av(*pending_av.pop(0))

                        if kt >= 2:
                            for _ in range(2):
                                if pending_po:
                                    pending_po.pop(0)()

                        if bb == 0 and kt % 4 == 2 and inject_pairs:
                            r0 = inject_pairs.pop(0)
                            emit_rt(r0)
                            emit_rt(r0 + 1)

                    for p, ep in pending_av:
                        fire_av(p, ep)

                    def make_evac(accA_, accB_, bb_, qoff_):  # noqa: unused-now-eager
                        # Evacuate accumulators with plain casts. The
                        # attention row-sums (65th accB row) are reshaped into
                        # per-partition columns via a DRAM bounce, then one
                        # wide reciprocal gives the 1/s scale column that the
                        # output-projection copy applies per token.
                        def evac():
                            outUa = sB.tile([128, QHW], BF16, name="outUa")
                            outUb = sB.tile([64, QHW], BF16, name="outUb")
                            s_row = sB.tile([1, QHW], F32, name="s_row")
                            for qc in range(QC):
                                nc.vector.tensor_copy(
                                    out=s_row[:, qc * 512:(qc + 1) * 512],
                                    in_=accB_[qc][64:65, :],
                                )
                            s_dram = sDram.tile([1, QHW], F32, name="s_dram")
                            nc.sync.dma_start(out=s_dram, in_=s_row)
                            for qc in range(QC):
                                nc.vector.tensor_copy(
                                    out=outUa[:, qc * 512:(qc + 1) * 512],
                                    in_=accA_[qc])
                                nc.vector.tensor_copy(
                                    out=outUb[:, qc * 512:(qc + 1) * 512],
                                    in_=accB_[qc][0:64, :])
                            rcol_raw = sB.tile([128, QHW // 128], F32,
                                               name="rcol_raw")
                            nc.sync.dma_start(
                                out=rcol_raw,
                                in_=s_dram.rearrange("one (t p) -> p (one t)",
                                                     p=128),
                            )
                            rcol = sB.tile([128, QHW // 128], F32, name="rcol")
                            nc.vector.reciprocal(out=rcol, in_=rcol_raw)
                            for t in range(QHW // 128):
                                for cc in range(CC):
                                    pending_po.append(functools.partial(
                                        emit_po, outUa, outUb, rcol,
                                        bb_, qoff_, t, cc))
                        return evac

                    make_evac(accA, accB, bb, qoff)()

                if bb == 0:
                    flush_tr()

            for fn in pending_po:
                fn(tail=True)

    nc.compile()
    return nc


_NC_CACHE = {}


def _get_nc(b=B, n=N, dim=DIM):
    key = (b, n, dim)
    if key not in _NC_CACHE:
        _NC_CACHE[key] = build_nc(b, n, dim)
    return _NC_CACHE[key]


def make_in_maps(x, attn_bias, w_qkv, w_out, g_q, g_k, g_v, n_cores=N_CORES):
    """Host-side shard + preprocess. Returns per-core input maps."""
    b, n, dim = x.shape
    bf = ml_dtypes.bfloat16
    xT = np.ascontiguousarray(
        x.reshape(b * n, dim).T).astype(bf)                      # [dim, b*n]
    kv_cols = np.ascontiguousarray(
        w_qkv[:, HEADS * DQK:]).astype(np.float32)               # [dim, 320]
    in_maps = []
    for c in range(n_cores):
        h = c % HEADS
        w_q_h = w_qkv[:, h * DQK:(h + 1) * DQK]
        w_all = np.concatenate([w_q_h, kv_cols], axis=1).astype(bf)  # [dim, 448]
        biasT = np.ascontiguousarray(
            attn_bias[:, h, :, :].transpose(0, 2, 1)).astype(bf)  # [b, keys, qrows]
        w_out_h = (w_out[h * DV:(h + 1) * DV, :]
                   * g_v[:, None].astype(np.float32)).astype(bf)  # [dv, dim]
        in_maps.append({
            "xT": xT,
            "w_all": w_all,
            "biasT": biasT,
            "w_out": w_out_h,
            "gq": (g_q * SCALE).astype(np.float32).reshape(DQK, 1),
            "gk": g_k.astype(np.float32).reshape(DQK, 1),
        })
    return in_maps


def kernel(x, attn_bias, w_qkv, w_out, g_q, g_k, g_v):
    x = np.asarray(x, dtype=np.float32)
    attn_bias = np.asarray(attn_bias, dtype=np.float32)
    w_qkv = np.asarray(w_qkv, dtype=np.float32)
    w_out = np.asarray(w_out, dtype=np.float32)
    g_q = np.asarray(g_q, dtype=np.float32)
    g_k = np.asarray(g_k, dtype=np.float32)
    g_v = np.asarray(g_v, dtype=np.float32)

    b, n, dim = x.shape
    nc = _get_nc(b, n, dim)
    in_maps = make_in_maps(x, attn_bias, w_qkv, w_out, g_q, g_k, g_v)
    res = run_bass_kernel_spmd(nc, in_maps, core_ids=list(range(N_CORES)),
                               trace=bool(os.environ.get("KERNEL_TRACE")))
    _LAST_STATS["exec_time_ns"] = res.exec_time_ns
    _LAST_STATS["mean_exec_time_ns"] = res.mean_exec_time_ns
    _LAST_STATS["res"] = res
    out = np.zeros((b, n, dim), dtype=np.float32)
    for c in range(N_CORES):
        out += res.results[c]["out"].astype(np.float32)
    return out



# revision 40
# speedup vs baseline: 1.1701x; 1.0533x over previous
"""Trainium2 Bass kernel for multi-query attention with tanh-clamped softmax.

Sharding: tensor-parallel over the 8 query heads (one head per core, both
batches). K/V projections are small and replicated. The output projection is
row-parallel (each core computes a full-shape partial); the host sums the 8
partials (the unshard step for row-parallel tensor parallelism).

All matmuls run in bf16 with fp32 PSUM accumulation; layernorm statistics and
softmax normalization are computed in fp32.
"""

import os
import sys

sys.path.insert(0, "/opt/trn_rl_repo")

import numpy as np
import ml_dtypes

import concourse.bass as bass
import concourse.tile as tile
from concourse import bacc, mybir
from concourse.bass_utils import run_bass_kernel_spmd
from concourse.masks import make_identity

F32 = mybir.dt.float32
BF16 = mybir.dt.bfloat16
F8 = mybir.dt.float8e4
AF = mybir.ActivationFunctionType
ALU = mybir.AluOpType

HEADS = 8
DQK = 128
DV = 192
SCALE = 64 ** -0.5
CLAMP = 5.0
EPS = 1e-5

B = 2
N = 2048
DIM = 1536
N_CORES = 8

AV_FP8 = os.environ.get("AV_FP8", "1") == "1"

_LAST_STATS = {}


def build_nc(b=B, n=N, dim=DIM):
    """Build the per-core Bass graph. All cores run the same graph (SPMD)."""
    assert dim % 128 == 0 and n % 512 == 0
    DIMT = dim // 128          # dim tiles (contraction for projections)
    RT_PER_B = n // 128        # row tiles per batch
    KT = n // 128              # key tiles per batch
    QH = n // 1024 if n >= 1024 else 1
    QHW = n // QH              # qrows per qhalf pass
    QC = QHW // 512            # 512-wide q chunks per qhalf
    CC = dim // 512            # output column chunks
    WCOLS = DQK + DQK + DV     # 448

    nc = bacc.Bacc("TRN2", target_bir_lowering=False)

    xT = nc.declare_dram_parameter("xT", [dim, b * n], BF16, isOutput=False)
    w_all = nc.declare_dram_parameter("w_all", [dim, WCOLS], BF16, isOutput=False)
    biasT = nc.declare_dram_parameter("biasT", [b, n, n], BF16, isOutput=False)
    w_out = nc.declare_dram_parameter("w_out", [DV, dim], BF16, isOutput=False)
    gq = nc.declare_dram_parameter("gq", [DQK, 1], F32, isOutput=False)
    gk = nc.declare_dram_parameter("gk", [DQK, 1], F32, isOutput=False)
    out = nc.declare_dram_parameter("out", [b, n, dim], BF16, isOutput=True)

    with tile.TileContext(nc) as tc:
        with (
            tc.tile_pool(name="const", bufs=1) as const,
            tc.tile_pool(name="big", bufs=1) as big,
            tc.tile_pool(name="stA", bufs=5) as sA,
            tc.tile_pool(name="stB", bufs=3) as sB,
            tc.tile_pool(name="biasp", bufs=6) as sBias,
            tc.tile_pool(name="expp", bufs=4) as sE,
            tc.tile_pool(name="dramp", bufs=3, space="DRAM") as sDram,
            tc.tile_pool(name="work_ps", bufs=4, space="PSUM") as psW,
            tc.tile_pool(name="acc_ps", bufs=1, space="PSUM") as psAcc,
        ):
            # ---------------- constants ----------------
            # split along dt so the first projection matmuls can start after
            # half the weight/x data has landed
            DTH = DIMT // 2
            w_all_sb = const.tile([128, DIMT, WCOLS], BF16)
            w_all_r = w_all.rearrange("(t p) c -> p t c", p=128)
            nc.scalar.dma_start(out=w_all_sb[:, :DTH, :], in_=w_all_r[:, :DTH, :])
            nc.scalar.dma_start(out=w_all_sb[:, DTH:, :], in_=w_all_r[:, DTH:, :])
            gq_sb = const.tile([128, 1], F32)
            nc.scalar.dma_start(out=gq_sb, in_=gq[:, :])
            gk_sb = const.tile([128, 1], F32)
            nc.scalar.dma_start(out=gk_sb, in_=gk[:, :])
            ident = const.tile([128, 128], BF16)
            make_identity(nc, ident)
            eps_sb = const.tile([128, 1], F32)
            nc.vector.memset(eps_sb, EPS)

            # ---------------- resident activations ----------------
            NXC = 8                      # x chunks (columns of xT), streamed
            XCW = (b * n) // NXC
            xTr = xT.rearrange("(t p) r -> p t r", p=128)

            qT_sb = [big.tile([128, n], BF16, name=f"qT{bb}") for bb in range(b)]
            kT_sb = [big.tile([128, n], BF16, name=f"kT{bb}") for bb in range(b)]
            # kt-stride padded to 208 B (16B-aligned) for DoubleRow ldweights
            VDT = F8 if AV_FP8 else BF16
            v_sb = [big.tile([128, KT, 208], VDT, name=f"v{bb}") for bb in range(b)]
            for bb in range(b):
                nc.vector.memset(v_sb[bb][:, :, DV:DV + 1], 1.0)

            # ---------------- stage A: QKV projection + LN + transpose ----------------
            # Transposes are emitted with a 2-iteration skew so the PE queue
            # never blocks on the LN chain of the current row-tile. Gamma
            # (and SCALE for q) is applied on ScalarE during the PSUM->SBUF
            # copy of each transposed tile.
            RT_PER_XC = XCW // 128
            pending_tr = []

            def emit_tr(bb_, ktile_, qn_, kn_, on_dve=False):
                qtp = psW.tile([128, 512], BF16, name="qtp", tag="w")[:, :128]
                nc.tensor.transpose(qtp, qn_, ident)
                ktp = psW.tile([128, 512], BF16, name="ktp", tag="w")[:, :128]
                nc.tensor.transpose(ktp, kn_, ident)
                qdst = qT_sb[bb_][:, ktile_ * 128:(ktile_ + 1) * 128]
                kdst = kT_sb[bb_][:, ktile_ * 128:(ktile_ + 1) * 128]
                if on_dve:
                    # bf16 PSUM src -> 2x mode; keeps the evac off the busy
                    # ACT FIFO when pairs are injected into the attention loop
                    nc.vector.tensor_scalar_mul(out=qdst, in0=qtp, scalar1=gq_sb)
                    nc.vector.tensor_scalar_mul(out=kdst, in0=ktp, scalar1=gk_sb)
                else:
                    nc.scalar.activation(out=qdst, in_=qtp, func=AF.Copy,
                                         scale=gq_sb)
                    nc.scalar.activation(out=kdst, in_=ktp, func=AF.Copy,
                                         scale=gk_sb)

            _rt_state = {"mvp": None, "held": None}
            _xt_chunks = {}

            def load_xt(xc, engine=None):
                eng = engine or nc.sync
                xt_sb = sA.tile([128, DIMT, XCW], BF16, name="xt_sb", tag="xt")
                # chunk 0 gates the very first matmuls: quarter it so the
                # first accumulation can start after ~0.4 MB lands
                nsplit = 4 if xc == 0 else 2
                step = DIMT // nsplit
                for s in range(nsplit):
                    eng.dma_start(
                        out=xt_sb[:, s * step:(s + 1) * step, :],
                        in_=xTr[:, s * step:(s + 1) * step,
                                xc * XCW:(xc + 1) * XCW],
                    )
                _xt_chunks[xc] = xt_sb

            def emit_rt(rt):
                bb = rt // RT_PER_B
                ktile = rt % RT_PER_B
                xc = rt // RT_PER_XC
                sub = rt % RT_PER_XC
                xoff = sub * 128
                if sub == 0 and xc not in _xt_chunks:
                    load_xt(xc)
                xt_sb = _xt_chunks[xc]

                qkv_ps = psW.tile([128, 512], F32, name="qkv_ps", tag="w")[:, :WCOLS]
                for dt_ in range(DIMT):
                    nc.tensor.matmul(
                        qkv_ps,
                        lhsT=xt_sb[:, dt_, xoff:xoff + 128],
                        rhs=w_all_sb[:, dt_, :],
                        start=(dt_ == 0),
                        stop=(dt_ == DIMT - 1),
                    )
                # single fast copy releases the PSUM slot; the LN chain (which
                # can sit behind the ACT/DVE backlog) reads the SBUF copy
                qkv_sb = sA.tile([128, WCOLS], F32, name="qkv_sb")
                nc.vector.tensor_copy(out=qkv_sb, in_=qkv_ps)
                if len(pending_tr) >= 2:
                    emit_tr(*pending_tr.pop(0))

                # layernorm stats for the three segments (q, k, v).
                # Stats for pairs of row-tiles share one mv tile so the
                # sqrt+reciprocal run once per pair (both have ~800ns fixed
                # cost); normalization is emitted on the odd row-tile.
                segs = [(0, DQK), (DQK, DQK), (2 * DQK, DV)]
                par = rt % 2
                if par == 0:
                    _rt_state["mvp"] = sA.tile([128, 2, 3, 2], F32, name="mvp")
                mvp = _rt_state["mvp"]
                stats = sA.tile([128, 3, 6], F32, name="stats")
                for si, (off, w) in enumerate(segs):
                    nc.vector.bn_stats(out=stats[:, si, :], in_=qkv_sb[:, off:off + w])
                    nc.vector.bn_aggr(out=mvp[:, par, si, :], in_=stats[:, si, :])
                if par == 0:
                    _rt_state["held"] = (bb, ktile, qkv_sb)
                    return
                held = _rt_state["held"]
                rstd = sA.tile([128, 2, 3], F32, name="rstd")
                nc.scalar.activation(rstd, mvp[:, :, :, 1], AF.Sqrt, bias=eps_sb)
                nc.vector.reciprocal(out=rstd, in_=rstd)

                for pp, (bb_, ktile_, ps_) in enumerate([held, (bb, ktile, qkv_sb)]):
                    qn = sA.tile([128, 128], BF16, name="qn")
                    nc.vector.tensor_scalar(
                        out=qn, in0=ps_[:, 0:DQK],
                        scalar1=mvp[:, pp, 0, 0:1], scalar2=rstd[:, pp, 0:1],
                        op0=ALU.subtract, op1=ALU.mult,
                    )
                    kn = sA.tile([128, 128], BF16, name="kn")
                    nc.vector.tensor_scalar(
                        out=kn, in0=ps_[:, DQK:2 * DQK],
                        scalar1=mvp[:, pp, 1, 0:1], scalar2=rstd[:, pp, 1:2],
                        op0=ALU.subtract, op1=ALU.mult,
                    )
                    nc.vector.tensor_scalar(
                        out=v_sb[bb_][:, ktile_, 0:DV], in0=ps_[:, 2 * DQK:WCOLS],
                        scalar1=mvp[:, pp, 2, 0:1], scalar2=rstd[:, pp, 2:3],
                        op0=ALU.subtract, op1=ALU.mult,
                    )
                    pending_tr.append((bb_, ktile_, qn, kn, bb_ == 1))

            def flush_tr():
                while pending_tr:
                    emit_tr(*pending_tr.pop(0))

            # batch 0's projection runs standalone; batch 1's row-tile pairs
            # are injected into batch 0's attention loop below so the PE
            # queue stays dense across the phase boundary.
            for rt in range(RT_PER_B):
                emit_rt(rt)
            flush_tr()
            # prefetch batch 1's x chunks on the (idle) scalar HWDGE queue so
            # the injected projection matmuls never block the PE FIFO on DMA
            for xc in range(RT_PER_B // RT_PER_XC, (b * RT_PER_B) // RT_PER_XC):
                load_xt(xc, engine=nc.scalar)
            inject_pairs = list(range(RT_PER_B, b * RT_PER_B, 2))

            # ---------------- stage B: attention + output projection ----------------
            # attn@v matmuls are emitted one kt-iteration behind their sim so
            # the PE never blocks on the DVE->ACT->ACT chain; the previous
            # qhalf's output projection is drip-fed into the kt loop.
            # w_out is loaded here (not with the other constants) so the head
            # DMA queue serves stage A's x/w tiles first.
            w_out_a = const.tile([128, dim], BF16)
            nc.sync.dma_start(out=w_out_a, in_=w_out[0:128, :])
            w_out_b = const.tile([64, dim], BF16)
            nc.sync.dma_start(out=w_out_b, in_=w_out[128:192, :])

            def emit_po(outUa_, outUb_, rcol_, bb_, qoff_, t, cc, tail=False):
                po = psW.tile([128, 512], F32, name="po", tag="w")
                nc.tensor.matmul(
                    po,
                    lhsT=outUa_[:, t * 128:(t + 1) * 128],
                    rhs=w_out_a[:, cc * 512:(cc + 1) * 512],
                    start=True, stop=False,
                )
                nc.tensor.matmul(
                    po,
                    lhsT=outUb_[:, t * 128:(t + 1) * 128],
                    rhs=w_out_b[:, cc * 512:(cc + 1) * 512],
                    start=False, stop=True,
                )
                po_sb = sB.tile([128, 512], BF16, name="po_sb")
                on_act = (t * CC + cc) % 2 == 1 if tail else (t * CC + cc) % 3 == 2
                if on_act:
                    nc.scalar.activation(
                        out=po_sb, in_=po, func=AF.Copy, scale=rcol_[:, t:t + 1])
                else:
                    nc.vector.tensor_scalar_mul(
                        out=po_sb, in0=po, scalar1=rcol_[:, t:t + 1])
                nc.sync.dma_start(
                    out=out[bb_, qoff_ + t * 128: qoff_ + (t + 1) * 128,
                            cc * 512:(cc + 1) * 512],
                    in_=po_sb,
                )

            import functools

            KT2 = KT // 2
            pending_po = []
            pending_evac = []
            for bb in range(b):
                for qh in range(QH):
                    qoff = qh * QHW
                    accA = [psAcc.tile([128, 512], F32, name=f"accA{qc}") for qc in range(QC)]
                    accB = [psAcc.tile([65, 512], F32, name=f"accB{qc}") for qc in range(QC)]

                    def fire_av(p, ep):
                        if AV_FP8:
                            # fp8 DoubleRow: one matmul covers key-tiles 2p, 2p+1.
                            for qc in range(QC):
                                nc.tensor.matmul(
                                    accA[qc],
                                    lhsT=v_sb[bb][:, 2 * p:2 * p + 2, 0:128],
                                    rhs=ep[:, :, qc * 512:(qc + 1) * 512],
                                    start=(p == 0), stop=(p == KT2 - 1),
                                    perf_mode=mybir.MatmulPerfMode.DoubleRow,
                                )
                                nc.tensor.matmul(
                                    accB[qc],
                                    lhsT=v_sb[bb][:, 2 * p:2 * p + 2, 128:DV + 1],
                                    rhs=ep[:, :, qc * 512:(qc + 1) * 512],
                                    start=(p == 0), stop=(p == KT2 - 1),
                                    perf_mode=mybir.MatmulPerfMode.DoubleRow,
                                )
                        else:
                            for s in range(2):
                                pkt = 2 * p + s
                                for qc in range(QC):
                                    nc.tensor.matmul(
                                        accA[qc],
                                        lhsT=v_sb[bb][:, pkt, 0:128],
                                        rhs=ep[:, s, qc * 512:(qc + 1) * 512],
                                        start=(pkt == 0), stop=(pkt == KT - 1),
                                    )
                                    nc.tensor.matmul(
                                        accB[qc],
                                        lhsT=v_sb[bb][:, pkt, 128:DV + 1],
                                        rhs=ep[:, s, qc * 512:(qc + 1) * 512],
                                        start=(pkt == 0), stop=(pkt == KT - 1),
                                    )

                    pending_av = []
                    e_pair = None
                    for kt in range(KT):
                        th_sb = sB.tile([128, QHW], F32, name="th_sb")
                        for qc in range(QC):
                            bias_sb = sBias.tile([128, 512], BF16, name="bias_sb")
                            nc.sync.dma_start(
                                out=bias_sb,
                                in_=biasT[bb, kt * 128:(kt + 1) * 128,
                                          qoff + qc * 512: qoff + (qc + 1) * 512],
                            )
                            sim_ps = psW.tile([128, 512], F32, name="sim_ps", tag="w")
                            nc.tensor.matmul(
                                sim_ps,
                                lhsT=kT_sb[bb][:, kt * 128:(kt + 1) * 128],
                                rhs=qT_sb[bb][:, qoff + qc * 512: qoff + (qc + 1) * 512],
                                start=True, stop=False,
                            )
                            # bias-add on the PE: identity matmul accumulates
                            # the bias tile into the sim PSUM bank, keeping
                            # the sim->tanh chain off the (FIFO-ordered) DVE.
                            nc.tensor.matmul(
                                sim_ps,
                                lhsT=ident,
                                rhs=bias_sb,
                                start=False, stop=True,
                            )
                            nc.scalar.activation(
                                th_sb[:, qc * 512:(qc + 1) * 512], sim_ps,
                                AF.Tanh, scale=1.0 / CLAMP)
                        if kt % 2 == 0:
                            e_pair = sE.tile([128, 2, QHW], VDT, name="e_pair")
                        nc.scalar.activation(e_pair[:, kt % 2, :], th_sb,
                                             AF.Exp, scale=CLAMP)
                        if kt % 2 == 1:
                            pending_av.append((kt // 2, e_pair))

                        if len(pending_av) >= 3:
                            fire_av(*pending_av.pop(0))

                        if kt >= 2:
                            for _ in range(2):
                                if pending_po:
                                    pending_po.pop(0)()

                        if bb == 0 and kt % 4 == 2 and inject_pairs:
                            r0 = inject_pairs.pop(0)
                            emit_rt(r0)
                            emit_rt(r0 + 1)

                    for p, ep in pending_av:
                        fire_av(p, ep)

                    def make_evac(accA_, accB_, bb_, qoff_):  # noqa: unused-now-eager
                        # Evacuate accumulators with plain casts. The
                        # attention row-sums (65th accB row) are reshaped into
                        # per-partition columns via a DRAM bounce, then one
                        # wide reciprocal gives the 1/s scale column that the
                        # output-projection copy applies per token.
                        def evac():
                            outUa = sB.tile([128, QHW], BF16, name="outUa")
                            outUb = sB.tile([64, QHW], BF16, name="outUb")
                            s_row = sB.tile([1, QHW], F32, name="s_row")
                            for qc in range(QC):
                                nc.vector.tensor_copy(
                                    out=s_row[:, qc * 512:(qc + 1) * 512],
                                    in_=accB_[qc][64:65, :],
                                )
                            s_dram = sDram.tile([1, QHW], F32, name="s_dram")
                            nc.sync.dma_start(out=s_dram, in_=s_row)
                            for qc in range(QC):
                                nc.vector.tensor_copy(
                                    out=outUa[:, qc * 512:(qc + 1) * 512],
                                    in_=accA_[qc])
                                nc.vector.tensor_copy(
                                    out=outUb[:, qc * 512:(qc + 1) * 512],
                                    in_=accB_[qc][0:64, :])
                            rcol_raw = sB.tile([128, QHW // 128], F32,
                                               name="rcol_raw")
                            nc.sync.dma_start(
                                out=rcol_raw,
                                in_=s_dram.rearrange("one (t p) -> p (one t)",
                                                     p=128),
                            )
                            rcol = sB.tile([128, QHW // 128], F32, name="rcol")
                            nc.vector.reciprocal(out=rcol, in_=rcol_raw)
                            for t in range(QHW // 128):
                                for cc in range(CC):
                                    pending_po.append(functools.partial(
                                        emit_po, outUa, outUb, rcol,
                                        bb_, qoff_, t, cc))
                        return evac

                    make_evac(accA, accB, bb, qoff)()

                if bb == 0:
                    flush_tr()

            for fn in pending_po:
                fn(tail=True)

    nc.compile()
    return nc


_NC_CACHE = {}


def _get_nc(b=B, n=N, dim=DIM):
    key = (b, n, dim)
    if key not in _NC_CACHE:
        _NC_CACHE[key] = build_nc(b, n, dim)
    return _NC_CACHE[key]


def make_in_maps(x, attn_bias, w_qkv, w_out, g_q, g_k, g_v, n_cores=N_CORES):
    """Host-side shard + preprocess. Returns per-core input maps."""
    b, n, dim = x.shape
    bf = ml_dtypes.bfloat16
    xT = np.ascontiguousarray(
        x.reshape(b * n, dim).T).astype(bf)                      # [dim, b*n]
    kv_cols = np.ascontiguousarray(
        w_qkv[:, HEADS * DQK:]).astype(np.float32)               # [dim, 320]
    in_maps = []
    for c in range(n_cores):
        h = c % HEADS
        w_q_h = w_qkv[:, h * DQK:(h + 1) * DQK]
        w_all = np.concatenate([w_q_h, kv_cols], axis=1).astype(bf)  # [dim, 448]
        biasT = np.ascontiguousarray(
            attn_bias[:, h, :, :].transpose(0, 2, 1)).astype(bf)  # [b, keys, qrows]
        w_out_h = (w_out[h * DV:(h + 1) * DV, :]
                   * g_v[:, None].astype(np.float32)).astype(bf)  # [dv, dim]
        in_maps.append({
            "xT": xT,
            "w_all": w_all,
            "biasT": biasT,
            "w_out": w_out_h,
            "gq": (g_q * SCALE).astype(np.float32).reshape(DQK, 1),
            "gk": g_k.astype(np.float32).reshape(DQK, 1),
        })
    return in_maps


def kernel(x, attn_bias, w_qkv, w_out, g_q, g_k, g_v):
    x = np.asarray(x, dtype=np.float32)
    attn_bias = np.asarray(attn_bias, dtype=np.float32)
    w_qkv = np.asarray(w_qkv, dtype=np.float32)
    w_out = np.asarray(w_out, dtype=np.float32)
    g_q = np.asarray(g_q, dtype=np.float32)
    g_k = np.asarray(g_k, dtype=np.float32)
    g_v = np.asarray(g_v, dtype=np.float32)

    b, n, dim = x.shape
    nc = _get_nc(b, n, dim)
    in_maps = make_in_maps(x, attn_bias, w_qkv, w_out, g_q, g_k, g_v)
    res = run_bass_kernel_spmd(nc, in_maps, core_ids=list(range(N_CORES)),
                               trace=bool(os.environ.get("KERNEL_TRACE")))
    _LAST_STATS["exec_time_ns"] = res.exec_time_ns
    _LAST_STATS["mean_exec_time_ns"] = res.mean_exec_time_ns
    _LAST_STATS["res"] = res
    out = np.zeros((b, n, dim), dtype=np.float32)
    for c in range(N_CORES):
        out += res.results[c]["out"].astype(np.float32)
    return out



# revision 41
# speedup vs baseline: 1.1807x; 1.0090x over previous
"""Trainium2 Bass kernel for multi-query attention with tanh-clamped softmax.

Sharding: tensor-parallel over the 8 query heads (one head per core, both
batches). K/V projections are small and replicated. The output projection is
row-parallel (each core computes a full-shape partial); the host sums the 8
partials (the unshard step for row-parallel tensor parallelism).

All matmuls run in bf16 with fp32 PSUM accumulation; layernorm statistics and
softmax normalization are computed in fp32.
"""

import os
import sys

sys.path.insert(0, "/opt/trn_rl_repo")

import numpy as np
import ml_dtypes

import concourse.bass as bass
import concourse.tile as tile
from concourse import bacc, mybir
from concourse.bass_utils import run_bass_kernel_spmd
from concourse.masks import make_identity

F32 = mybir.dt.float32
BF16 = mybir.dt.bfloat16
F8 = mybir.dt.float8e4
AF = mybir.ActivationFunctionType
ALU = mybir.AluOpType

HEADS = 8
DQK = 128
DV = 192
SCALE = 64 ** -0.5
CLAMP = 5.0
EPS = 1e-5

B = 2
N = 2048
DIM = 1536
N_CORES = 8

AV_FP8 = os.environ.get("AV_FP8", "1") == "1"

_LAST_STATS = {}


def build_nc(b=B, n=N, dim=DIM):
    """Build the per-core Bass graph. All cores run the same graph (SPMD)."""
    assert dim % 128 == 0 and n % 512 == 0
    DIMT = dim // 128          # dim tiles (contraction for projections)
    RT_PER_B = n // 128        # row tiles per batch
    KT = n // 128              # key tiles per batch
    QH = n // 1024 if n >= 1024 else 1
    QHW = n // QH              # qrows per qhalf pass
    QC = QHW // 512            # 512-wide q chunks per qhalf
    CC = dim // 512            # output column chunks
    WCOLS = DQK + DQK + DV     # 448

    nc = bacc.Bacc("TRN2", target_bir_lowering=False)

    xT = nc.declare_dram_parameter("xT", [dim, b * n], BF16, isOutput=False)
    w_all = nc.declare_dram_parameter("w_all", [dim, WCOLS], BF16, isOutput=False)
    biasT = nc.declare_dram_parameter("biasT", [b, n, n], BF16, isOutput=False)
    w_out = nc.declare_dram_parameter("w_out", [DV, dim], BF16, isOutput=False)
    gq = nc.declare_dram_parameter("gq", [DQK, 1], F32, isOutput=False)
    gk = nc.declare_dram_parameter("gk", [DQK, 1], F32, isOutput=False)
    out = nc.declare_dram_parameter("out", [b, n, dim], BF16, isOutput=True)

    with tile.TileContext(nc) as tc:
        with (
            tc.tile_pool(name="const", bufs=1) as const,
            tc.tile_pool(name="big", bufs=1) as big,
            tc.tile_pool(name="stA", bufs=5) as sA,
            tc.tile_pool(name="stB", bufs=3) as sB,
            tc.tile_pool(name="biasp", bufs=6) as sBias,
            tc.tile_pool(name="expp", bufs=4) as sE,
            tc.tile_pool(name="dramp", bufs=3, space="DRAM") as sDram,
            tc.tile_pool(name="work_ps", bufs=4, space="PSUM") as psW,
            tc.tile_pool(name="acc_ps", bufs=1, space="PSUM") as psAcc,
        ):
            # ---------------- constants ----------------
            # split along dt so the first projection matmuls can start after
            # half the weight/x data has landed
            DTH = DIMT // 2
            w_all_sb = const.tile([128, DIMT, WCOLS], BF16)
            w_all_r = w_all.rearrange("(t p) c -> p t c", p=128)
            nc.scalar.dma_start(out=w_all_sb[:, :DTH, :], in_=w_all_r[:, :DTH, :])
            nc.scalar.dma_start(out=w_all_sb[:, DTH:, :], in_=w_all_r[:, DTH:, :])
            gq_sb = const.tile([128, 1], F32)
            nc.scalar.dma_start(out=gq_sb, in_=gq[:, :])
            gk_sb = const.tile([128, 1], F32)
            nc.scalar.dma_start(out=gk_sb, in_=gk[:, :])
            ident = const.tile([128, 128], BF16)
            make_identity(nc, ident)
            eps_sb = const.tile([128, 1], F32)
            nc.vector.memset(eps_sb, EPS)
            # HAM pre-warm: ~3.5 us of dummy PE activity during the DMA head
            # so the first real matmuls start at the full 2.4 GHz clock
            warm_sb = const.tile([128, 128], BF16)
            nc.vector.memset(warm_sb, 0.0)
            warm_ps = psW.tile([128, 512], F32, name="warm_ps", tag="w")
            for _ in range(30):
                nc.tensor.matmul(warm_ps[:, :128], lhsT=warm_sb, rhs=warm_sb,
                                 start=True, stop=True)

            # ---------------- resident activations ----------------
            NXC = 8                      # x chunks (columns of xT), streamed
            XCW = (b * n) // NXC
            xTr = xT.rearrange("(t p) r -> p t r", p=128)

            qT_sb = [big.tile([128, n], BF16, name=f"qT{bb}") for bb in range(b)]
            kT_sb = [big.tile([128, n], BF16, name=f"kT{bb}") for bb in range(b)]
            # kt-stride padded to 208 B (16B-aligned) for DoubleRow ldweights
            VDT = F8 if AV_FP8 else BF16
            v_sb = [big.tile([128, KT, 208], VDT, name=f"v{bb}") for bb in range(b)]
            for bb in range(b):
                nc.vector.memset(v_sb[bb][:, :, DV:DV + 1], 1.0)

            # ---------------- stage A: QKV projection + LN + transpose ----------------
            # Transposes are emitted with a 2-iteration skew so the PE queue
            # never blocks on the LN chain of the current row-tile. Gamma
            # (and SCALE for q) is applied on ScalarE during the PSUM->SBUF
            # copy of each transposed tile.
            RT_PER_XC = XCW // 128
            pending_tr = []

            def emit_tr(bb_, ktile_, qn_, kn_, on_dve=False):
                qtp = psW.tile([128, 512], BF16, name="qtp", tag="w")[:, :128]
                nc.tensor.transpose(qtp, qn_, ident)
                ktp = psW.tile([128, 512], BF16, name="ktp", tag="w")[:, :128]
                nc.tensor.transpose(ktp, kn_, ident)
                qdst = qT_sb[bb_][:, ktile_ * 128:(ktile_ + 1) * 128]
                kdst = kT_sb[bb_][:, ktile_ * 128:(ktile_ + 1) * 128]
                if on_dve:
                    # bf16 PSUM src -> 2x mode; keeps the evac off the busy
                    # ACT FIFO when pairs are injected into the attention loop
                    nc.vector.tensor_scalar_mul(out=qdst, in0=qtp, scalar1=gq_sb)
                    nc.vector.tensor_scalar_mul(out=kdst, in0=ktp, scalar1=gk_sb)
                else:
                    nc.scalar.activation(out=qdst, in_=qtp, func=AF.Copy,
                                         scale=gq_sb)
                    nc.scalar.activation(out=kdst, in_=ktp, func=AF.Copy,
                                         scale=gk_sb)

            _rt_state = {"mvp": None, "held": None}
            _xt_chunks = {}

            def load_xt(xc, engine=None):
                eng = engine or nc.sync
                xt_sb = sA.tile([128, DIMT, XCW], BF16, name="xt_sb", tag="xt")
                # chunk 0 gates the very first matmuls: quarter it so the
                # first accumulation can start after ~0.4 MB lands
                nsplit = 4 if xc == 0 else 2
                step = DIMT // nsplit
                for s in range(nsplit):
                    eng.dma_start(
                        out=xt_sb[:, s * step:(s + 1) * step, :],
                        in_=xTr[:, s * step:(s + 1) * step,
                                xc * XCW:(xc + 1) * XCW],
                    )
                _xt_chunks[xc] = xt_sb

            def emit_rt(rt):
                bb = rt // RT_PER_B
                ktile = rt % RT_PER_B
                xc = rt // RT_PER_XC
                sub = rt % RT_PER_XC
                xoff = sub * 128
                if sub == 0 and xc not in _xt_chunks:
                    load_xt(xc)
                xt_sb = _xt_chunks[xc]

                qkv_ps = psW.tile([128, 512], F32, name="qkv_ps", tag="w")[:, :WCOLS]
                for dt_ in range(DIMT):
                    nc.tensor.matmul(
                        qkv_ps,
                        lhsT=xt_sb[:, dt_, xoff:xoff + 128],
                        rhs=w_all_sb[:, dt_, :],
                        start=(dt_ == 0),
                        stop=(dt_ == DIMT - 1),
                    )
                # single fast copy releases the PSUM slot; the LN chain (which
                # can sit behind the ACT/DVE backlog) reads the SBUF copy
                qkv_sb = sA.tile([128, WCOLS], F32, name="qkv_sb")
                nc.vector.tensor_copy(out=qkv_sb, in_=qkv_ps)
                if len(pending_tr) >= 2:
                    emit_tr(*pending_tr.pop(0))

                # layernorm stats for the three segments (q, k, v).
                # Stats for pairs of row-tiles share one mv tile so the
                # sqrt+reciprocal run once per pair (both have ~800ns fixed
                # cost); normalization is emitted on the odd row-tile.
                segs = [(0, DQK), (DQK, DQK), (2 * DQK, DV)]
                par = rt % 2
                if par == 0:
                    _rt_state["mvp"] = sA.tile([128, 2, 3, 2], F32, name="mvp")
                mvp = _rt_state["mvp"]
                stats = sA.tile([128, 3, 6], F32, name="stats")
                for si, (off, w) in enumerate(segs):
                    nc.vector.bn_stats(out=stats[:, si, :], in_=qkv_sb[:, off:off + w])
                    nc.vector.bn_aggr(out=mvp[:, par, si, :], in_=stats[:, si, :])
                if par == 0:
                    _rt_state["held"] = (bb, ktile, qkv_sb)
                    return
                held = _rt_state["held"]
                rstd = sA.tile([128, 2, 3], F32, name="rstd")
                nc.scalar.activation(rstd, mvp[:, :, :, 1], AF.Sqrt, bias=eps_sb)
                nc.vector.reciprocal(out=rstd, in_=rstd)

                for pp, (bb_, ktile_, ps_) in enumerate([held, (bb, ktile, qkv_sb)]):
                    qn = sA.tile([128, 128], BF16, name="qn")
                    nc.vector.tensor_scalar(
                        out=qn, in0=ps_[:, 0:DQK],
                        scalar1=mvp[:, pp, 0, 0:1], scalar2=rstd[:, pp, 0:1],
                        op0=ALU.subtract, op1=ALU.mult,
                    )
                    kn = sA.tile([128, 128], BF16, name="kn")
                    nc.vector.tensor_scalar(
                        out=kn, in0=ps_[:, DQK:2 * DQK],
                        scalar1=mvp[:, pp, 1, 0:1], scalar2=rstd[:, pp, 1:2],
                        op0=ALU.subtract, op1=ALU.mult,
                    )
                    nc.vector.tensor_scalar(
                        out=v_sb[bb_][:, ktile_, 0:DV], in0=ps_[:, 2 * DQK:WCOLS],
                        scalar1=mvp[:, pp, 2, 0:1], scalar2=rstd[:, pp, 2:3],
                        op0=ALU.subtract, op1=ALU.mult,
                    )
                    pending_tr.append((bb_, ktile_, qn, kn, bb_ == 1))

            def flush_tr():
                while pending_tr:
                    emit_tr(*pending_tr.pop(0))

            # batch 0's projection runs standalone; batch 1's row-tile pairs
            # are injected into batch 0's attention loop below so the PE
            # queue stays dense across the phase boundary.
            for rt in range(RT_PER_B):
                emit_rt(rt)
            flush_tr()
            # prefetch batch 1's x chunks on the (idle) scalar HWDGE queue so
            # the injected projection matmuls never block the PE FIFO on DMA
            for xc in range(RT_PER_B // RT_PER_XC, (b * RT_PER_B) // RT_PER_XC):
                load_xt(xc, engine=nc.scalar)
            inject_pairs = list(range(RT_PER_B, b * RT_PER_B, 2))

            # ---------------- stage B: attention + output projection ----------------
            # attn@v matmuls are emitted one kt-iteration behind their sim so
            # the PE never blocks on the DVE->ACT->ACT chain; the previous
            # qhalf's output projection is drip-fed into the kt loop.
            # w_out is loaded here (not with the other constants) so the head
            # DMA queue serves stage A's x/w tiles first.
            w_out_a = const.tile([128, dim], BF16)
            nc.sync.dma_start(out=w_out_a, in_=w_out[0:128, :])
            w_out_b = const.tile([64, dim], BF16)
            nc.sync.dma_start(out=w_out_b, in_=w_out[128:192, :])

            def emit_po(outUa_, outUb_, rcol_, bb_, qoff_, t, cc, tail=False):
                po = psW.tile([128, 512], F32, name="po", tag="w")
                nc.tensor.matmul(
                    po,
                    lhsT=outUa_[:, t * 128:(t + 1) * 128],
                    rhs=w_out_a[:, cc * 512:(cc + 1) * 512],
                    start=True, stop=False,
                )
                nc.tensor.matmul(
                    po,
                    lhsT=outUb_[:, t * 128:(t + 1) * 128],
                    rhs=w_out_b[:, cc * 512:(cc + 1) * 512],
                    start=False, stop=True,
                )
                po_sb = sB.tile([128, 512], BF16, name="po_sb")
                on_act = (t * CC + cc) % 2 == 1 if tail else (t * CC + cc) % 3 == 2
                if on_act:
                    nc.scalar.activation(
                        out=po_sb, in_=po, func=AF.Copy, scale=rcol_[:, t:t + 1])
                else:
                    nc.vector.tensor_scalar_mul(
                        out=po_sb, in0=po, scalar1=rcol_[:, t:t + 1])
                nc.sync.dma_start(
                    out=out[bb_, qoff_ + t * 128: qoff_ + (t + 1) * 128,
                            cc * 512:(cc + 1) * 512],
                    in_=po_sb,
                )

            import functools

            KT2 = KT // 2
            pending_po = []
            pending_evac = []
            for bb in range(b):
                for qh in range(QH):
                    qoff = qh * QHW
                    accA = [psAcc.tile([128, 512], F32, name=f"accA{qc}") for qc in range(QC)]
                    accB = [psAcc.tile([65, 512], F32, name=f"accB{qc}") for qc in range(QC)]

                    def fire_av(p, ep):
                        if AV_FP8:
                            # fp8 DoubleRow: one matmul covers key-tiles 2p, 2p+1.
                            for qc in range(QC):
                                nc.tensor.matmul(
                                    accA[qc],
                                    lhsT=v_sb[bb][:, 2 * p:2 * p + 2, 0:128],
                                    rhs=ep[:, :, qc * 512:(qc + 1) * 512],
                                    start=(p == 0), stop=(p == KT2 - 1),
                                    perf_mode=mybir.MatmulPerfMode.DoubleRow,
                                )
                                nc.tensor.matmul(
                                    accB[qc],
                                    lhsT=v_sb[bb][:, 2 * p:2 * p + 2, 128:DV + 1],
                                    rhs=ep[:, :, qc * 512:(qc + 1) * 512],
                                    start=(p == 0), stop=(p == KT2 - 1),
                                    perf_mode=mybir.MatmulPerfMode.DoubleRow,
                                )
                        else:
                            for s in range(2):
                                pkt = 2 * p + s
                                for qc in range(QC):
                                    nc.tensor.matmul(
                                        accA[qc],
                                        lhsT=v_sb[bb][:, pkt, 0:128],
                                        rhs=ep[:, s, qc * 512:(qc + 1) * 512],
                                        start=(pkt == 0), stop=(pkt == KT - 1),
                                    )
                                    nc.tensor.matmul(
                                        accB[qc],
                                        lhsT=v_sb[bb][:, pkt, 128:DV + 1],
                                        rhs=ep[:, s, qc * 512:(qc + 1) * 512],
                                        start=(pkt == 0), stop=(pkt == KT - 1),
                                    )

                    pending_av = []
                    e_pair = None
                    for kt in range(KT):
                        th_sb = sB.tile([128, QHW], F32, name="th_sb")
                        for qc in range(QC):
                            bias_sb = sBias.tile([128, 512], BF16, name="bias_sb")
                            nc.sync.dma_start(
                                out=bias_sb,
                                in_=biasT[bb, kt * 128:(kt + 1) * 128,
                                          qoff + qc * 512: qoff + (qc + 1) * 512],
                            )
                            sim_ps = psW.tile([128, 512], F32, name="sim_ps", tag="w")
                            nc.tensor.matmul(
                                sim_ps,
                                lhsT=kT_sb[bb][:, kt * 128:(kt + 1) * 128],
                                rhs=qT_sb[bb][:, qoff + qc * 512: qoff + (qc + 1) * 512],
                                start=True, stop=False,
                            )
                            # bias-add on the PE: identity matmul accumulates
                            # the bias tile into the sim PSUM bank, keeping
                            # the sim->tanh chain off the (FIFO-ordered) DVE.
                            nc.tensor.matmul(
                                sim_ps,
                                lhsT=ident,
                                rhs=bias_sb,
                                start=False, stop=True,
                            )
                            nc.scalar.activation(
                                th_sb[:, qc * 512:(qc + 1) * 512], sim_ps,
                                AF.Tanh, scale=1.0 / CLAMP)
                        if kt % 2 == 0:
                            e_pair = sE.tile([128, 2, QHW], VDT, name="e_pair")
                        nc.scalar.activation(e_pair[:, kt % 2, :], th_sb,
                                             AF.Exp, scale=CLAMP)
                        if kt % 2 == 1:
                            pending_av.append((kt // 2, e_pair))

                        if len(pending_av) >= 3:
                            fire_av(*pending_av.pop(0))

                        if kt >= 2:
                            for _ in range(2):
                                if pending_po:
                                    pending_po.pop(0)()

                        if bb == 0 and kt % 4 == 2 and inject_pairs:
                            r0 = inject_pairs.pop(0)
                            emit_rt(r0)
                            emit_rt(r0 + 1)

                    for p, ep in pending_av:
                        fire_av(p, ep)

                    def make_evac(accA_, accB_, bb_, qoff_):  # noqa: unused-now-eager
                        # Evacuate accumulators with plain casts. The
                        # attention row-sums (65th accB row) are reshaped into
                        # per-partition columns via a DRAM bounce, then one
                        # wide reciprocal gives the 1/s scale column that the
                        # output-projection copy applies per token.
                        def evac():
                            outUa = sB.tile([128, QHW], BF16, name="outUa")
                            outUb = sB.tile([64, QHW], BF16, name="outUb")
                            s_row = sB.tile([1, QHW], F32, name="s_row")
                            for qc in range(QC):
                                nc.vector.tensor_copy(
                                    out=s_row[:, qc * 512:(qc + 1) * 512],
                                    in_=accB_[qc][64:65, :],
                                )
                            s_dram = sDram.tile([1, QHW], F32, name="s_dram")
                            nc.sync.dma_start(out=s_dram, in_=s_row)
                            for qc in range(QC):
                                nc.vector.tensor_copy(
                                    out=outUa[:, qc * 512:(qc + 1) * 512],
                                    in_=accA_[qc])
                                nc.vector.tensor_copy(
                                    out=outUb[:, qc * 512:(qc + 1) * 512],
                                    in_=accB_[qc][0:64, :])
                            rcol_raw = sB.tile([128, QHW // 128], F32,
                                               name="rcol_raw")
                            nc.sync.dma_start(
                                out=rcol_raw,
                                in_=s_dram.rearrange("one (t p) -> p (one t)",
                                                     p=128),
                            )
                            rcol = sB.tile([128, QHW // 128], F32, name="rcol")
                            nc.vector.reciprocal(out=rcol, in_=rcol_raw)
                            for t in range(QHW // 128):
                                for cc in range(CC):
                                    pending_po.append(functools.partial(
                                        emit_po, outUa, outUb, rcol,
                                        bb_, qoff_, t, cc))
                        return evac

                    make_evac(accA, accB, bb, qoff)()

                if bb == 0:
                    flush_tr()

            for fn in pending_po:
                fn(tail=True)

    nc.compile()
    return nc


_NC_CACHE = {}


def _get_nc(b=B, n=N, dim=DIM):
    key = (b, n, dim)
    if key not in _NC_CACHE:
        _NC_CACHE[key] = build_nc(b, n, dim)
    return _NC_CACHE[key]


def make_in_maps(x, attn_bias, w_qkv, w_out, g_q, g_k, g_v, n_cores=N_CORES):
    """Host-side shard + preprocess. Returns per-core input maps."""
    b, n, dim = x.shape
    bf = ml_dtypes.bfloat16
    xT = np.ascontiguousarray(
        x.reshape(b * n, dim).T).astype(bf)                      # [dim, b*n]
    kv_cols = np.ascontiguousarray(
        w_qkv[:, HEADS * DQK:]).astype(np.float32)               # [dim, 320]
    in_maps = []
    for c in range(n_cores):
        h = c % HEADS
        w_q_h = w_qkv[:, h * DQK:(h + 1) * DQK]
        w_all = np.concatenate([w_q_h, kv_cols], axis=1).astype(bf)  # [dim, 448]
        biasT = np.ascontiguousarray(
            attn_bias[:, h, :, :].transpose(0, 2, 1)).astype(bf)  # [b, keys, qrows]
        w_out_h = (w_out[h * DV:(h + 1) * DV, :]
                   * g_v[:, None].astype(np.float32)).astype(bf)  # [dv, dim]
        in_maps.append({
            "xT": xT,
            "w_all": w_all,
            "biasT": biasT,
            "w_out": w_out_h,
            "gq": (g_q * SCALE).astype(np.float32).reshape(DQK, 1),
            "gk": g_k.astype(np.float32).reshape(DQK, 1),
        })
    return in_maps


def kernel(x, attn_bias, w_qkv, w_out, g_q, g_k, g_v):
    x = np.asarray(x, dtype=np.float32)
    attn_bias = np.asarray(attn_bias, dtype=np.float32)
    w_qkv = np.asarray(w_qkv, dtype=np.float32)
    w_out = np.asarray(w_out, dtype=np.float32)
    g_q = np.asarray(g_q, dtype=np.float32)
    g_k = np.asarray(g_k, dtype=np.float32)
    g_v = np.asarray(g_v, dtype=np.float32)

    b, n, dim = x.shape
    nc = _get_nc(b, n, dim)
    in_maps = make_in_maps(x, attn_bias, w_qkv, w_out, g_q, g_k, g_v)
    res = run_bass_kernel_spmd(nc, in_maps, core_ids=list(range(N_CORES)),
                               trace=bool(os.environ.get("KERNEL_TRACE")))
    _LAST_STATS["exec_time_ns"] = res.exec_time_ns
    _LAST_STATS["mean_exec_time_ns"] = res.mean_exec_time_ns
    _LAST_STATS["res"] = res
    out = np.zeros((b, n, dim), dtype=np.float32)
    for c in range(N_CORES):
        out += res.results[c]["out"].astype(np.float32)
    return out



# revision 42
# speedup vs baseline: 1.1932x; 1.0106x over previous
"""Trainium2 Bass kernel for multi-query attention with tanh-clamped softmax.

Sharding: tensor-parallel over the 8 query heads (one head per core, both
batches). K/V projections are small and replicated. The output projection is
row-parallel (each core computes a full-shape partial); the host sums the 8
partials (the unshard step for row-parallel tensor parallelism).

All matmuls run in bf16 with fp32 PSUM accumulation; layernorm statistics and
softmax normalization are computed in fp32.
"""

import os
import sys

sys.path.insert(0, "/opt/trn_rl_repo")

import numpy as np
import ml_dtypes

import concourse.bass as bass
import concourse.tile as tile
from concourse import bacc, mybir
from concourse.bass_utils import run_bass_kernel_spmd
from concourse.masks import make_identity

F32 = mybir.dt.float32
BF16 = mybir.dt.bfloat16
F8 = mybir.dt.float8e4
AF = mybir.ActivationFunctionType
ALU = mybir.AluOpType

HEADS = 8
DQK = 128
DV = 192
SCALE = 64 ** -0.5
CLAMP = 5.0
EPS = 1e-5

B = 2
N = 2048
DIM = 1536
N_CORES = 8

AV_FP8 = os.environ.get("AV_FP8", "1") == "1"

_LAST_STATS = {}


def build_nc(b=B, n=N, dim=DIM):
    """Build the per-core Bass graph. All cores run the same graph (SPMD)."""
    assert dim % 128 == 0 and n % 512 == 0
    DIMT = dim // 128          # dim tiles (contraction for projections)
    RT_PER_B = n // 128        # row tiles per batch
    KT = n // 128              # key tiles per batch
    QH = n // 1024 if n >= 1024 else 1
    QHW = n // QH              # qrows per qhalf pass
    QC = QHW // 512            # 512-wide q chunks per qhalf
    CC = dim // 512            # output column chunks
    WCOLS = DQK + DQK + DV     # 448

    nc = bacc.Bacc("TRN2", target_bir_lowering=False)

    xT = nc.declare_dram_parameter("xT", [dim, b * n], BF16, isOutput=False)
    w_all = nc.declare_dram_parameter("w_all", [dim, WCOLS], BF16, isOutput=False)
    biasT = nc.declare_dram_parameter("biasT", [b, n, n], BF16, isOutput=False)
    w_out = nc.declare_dram_parameter("w_out", [DV, dim], BF16, isOutput=False)
    gq = nc.declare_dram_parameter("gq", [DQK, 1], F32, isOutput=False)
    gk = nc.declare_dram_parameter("gk", [DQK, 1], F32, isOutput=False)
    out = nc.declare_dram_parameter("out", [b, n, dim], BF16, isOutput=True)

    with tile.TileContext(nc) as tc:
        with (
            tc.tile_pool(name="const", bufs=1) as const,
            tc.tile_pool(name="big", bufs=1) as big,
            tc.tile_pool(name="stA", bufs=5) as sA,
            tc.tile_pool(name="stB", bufs=3) as sB,
            tc.tile_pool(name="biasp", bufs=6) as sBias,
            tc.tile_pool(name="expp", bufs=4) as sE,
            tc.tile_pool(name="dramp", bufs=3, space="DRAM") as sDram,
            tc.tile_pool(name="work_ps", bufs=4, space="PSUM") as psW,
            tc.tile_pool(name="acc_ps", bufs=1, space="PSUM") as psAcc,
        ):
            # ---------------- constants ----------------
            # split along dt so the first projection matmuls can start after
            # half the weight/x data has landed
            DTH = DIMT // 2
            w_all_sb = const.tile([128, DIMT, WCOLS], BF16)
            w_all_r = w_all.rearrange("(t p) c -> p t c", p=128)
            nc.scalar.dma_start(out=w_all_sb[:, :DTH, :], in_=w_all_r[:, :DTH, :])
            nc.scalar.dma_start(out=w_all_sb[:, DTH:, :], in_=w_all_r[:, DTH:, :])
            gq_sb = const.tile([128, 1], F32)
            nc.scalar.dma_start(out=gq_sb, in_=gq[:, :])
            gk_sb = const.tile([128, 1], F32)
            nc.scalar.dma_start(out=gk_sb, in_=gk[:, :])
            ident = const.tile([128, 128], BF16)
            make_identity(nc, ident)
            eps_sb = const.tile([128, 1], F32)
            nc.vector.memset(eps_sb, EPS)
            I32 = mybir.dt.int32
            magic_sb = const.tile([128, 2, 3], I32)
            nc.vector.memset(magic_sb, 0x5F3759DF)
            # HAM pre-warm: ~3.5 us of dummy PE activity during the DMA head
            # so the first real matmuls start at the full 2.4 GHz clock
            warm_sb = const.tile([128, 128], BF16)
            nc.vector.memset(warm_sb, 0.0)
            warm_ps = psW.tile([128, 512], F32, name="warm_ps", tag="w")
            for _ in range(30):
                nc.tensor.matmul(warm_ps[:, :128], lhsT=warm_sb, rhs=warm_sb,
                                 start=True, stop=True)

            # ---------------- resident activations ----------------
            NXC = 8                      # x chunks (columns of xT), streamed
            XCW = (b * n) // NXC
            xTr = xT.rearrange("(t p) r -> p t r", p=128)

            qT_sb = [big.tile([128, n], BF16, name=f"qT{bb}") for bb in range(b)]
            kT_sb = [big.tile([128, n], BF16, name=f"kT{bb}") for bb in range(b)]
            # kt-stride padded to 208 B (16B-aligned) for DoubleRow ldweights
            VDT = F8 if AV_FP8 else BF16
            v_sb = [big.tile([128, KT, 208], VDT, name=f"v{bb}") for bb in range(b)]
            for bb in range(b):
                nc.vector.memset(v_sb[bb][:, :, DV:DV + 1], 1.0)

            # ---------------- stage A: QKV projection + LN + transpose ----------------
            # Transposes are emitted with a 2-iteration skew so the PE queue
            # never blocks on the LN chain of the current row-tile. Gamma
            # (and SCALE for q) is applied on ScalarE during the PSUM->SBUF
            # copy of each transposed tile.
            RT_PER_XC = XCW // 128
            pending_tr = []

            def emit_tr(bb_, ktile_, qn_, kn_, on_dve=False):
                qtp = psW.tile([128, 512], BF16, name="qtp", tag="w")[:, :128]
                nc.tensor.transpose(qtp, qn_, ident)
                ktp = psW.tile([128, 512], BF16, name="ktp", tag="w")[:, :128]
                nc.tensor.transpose(ktp, kn_, ident)
                qdst = qT_sb[bb_][:, ktile_ * 128:(ktile_ + 1) * 128]
                kdst = kT_sb[bb_][:, ktile_ * 128:(ktile_ + 1) * 128]
                if on_dve:
                    # bf16 PSUM src -> 2x mode; keeps the evac off the busy
                    # ACT FIFO when pairs are injected into the attention loop
                    nc.vector.tensor_scalar_mul(out=qdst, in0=qtp, scalar1=gq_sb)
                    nc.vector.tensor_scalar_mul(out=kdst, in0=ktp, scalar1=gk_sb)
                else:
                    nc.scalar.activation(out=qdst, in_=qtp, func=AF.Copy,
                                         scale=gq_sb)
                    nc.scalar.activation(out=kdst, in_=ktp, func=AF.Copy,
                                         scale=gk_sb)

            _rt_state = {"mvp": None, "held": None}
            _xt_chunks = {}

            def load_xt(xc, engine=None):
                eng = engine or nc.sync
                xt_sb = sA.tile([128, DIMT, XCW], BF16, name="xt_sb", tag="xt")
                # chunk 0 gates the very first matmuls: quarter it so the
                # first accumulation can start after ~0.4 MB lands
                nsplit = 4 if xc == 0 else 2
                step = DIMT // nsplit
                for s in range(nsplit):
                    eng.dma_start(
                        out=xt_sb[:, s * step:(s + 1) * step, :],
                        in_=xTr[:, s * step:(s + 1) * step,
                                xc * XCW:(xc + 1) * XCW],
                    )
                _xt_chunks[xc] = xt_sb

            def emit_rt(rt):
                bb = rt // RT_PER_B
                ktile = rt % RT_PER_B
                xc = rt // RT_PER_XC
                sub = rt % RT_PER_XC
                xoff = sub * 128
                if sub == 0 and xc not in _xt_chunks:
                    load_xt(xc)
                xt_sb = _xt_chunks[xc]

                qkv_ps = psW.tile([128, 512], F32, name="qkv_ps", tag="w")[:, :WCOLS]
                for dt_ in range(DIMT):
                    nc.tensor.matmul(
                        qkv_ps,
                        lhsT=xt_sb[:, dt_, xoff:xoff + 128],
                        rhs=w_all_sb[:, dt_, :],
                        start=(dt_ == 0),
                        stop=(dt_ == DIMT - 1),
                    )
                # single fast copy releases the PSUM slot; the LN chain (which
                # can sit behind the ACT/DVE backlog) reads the SBUF copy
                qkv_sb = sA.tile([128, WCOLS], F32, name="qkv_sb")
                nc.vector.tensor_copy(out=qkv_sb, in_=qkv_ps)
                if len(pending_tr) >= 2:
                    emit_tr(*pending_tr.pop(0))

                # layernorm stats for the three segments (q, k, v).
                # Stats for pairs of row-tiles share one mv tile so the
                # sqrt+reciprocal run once per pair (both have ~800ns fixed
                # cost); normalization is emitted on the odd row-tile.
                segs = [(0, DQK), (DQK, DQK), (2 * DQK, DV)]
                par = rt % 2
                if par == 0:
                    _rt_state["mvp"] = sA.tile([128, 2, 3, 2], F32, name="mvp")
                mvp = _rt_state["mvp"]
                stats = sA.tile([128, 3, 6], F32, name="stats")
                for si, (off, w) in enumerate(segs):
                    nc.vector.bn_stats(out=stats[:, si, :], in_=qkv_sb[:, off:off + w])
                    nc.vector.bn_aggr(out=mvp[:, par, si, :], in_=stats[:, si, :])
                if par == 0:
                    _rt_state["held"] = (bb, ktile, qkv_sb)
                    return
                held = _rt_state["held"]
                rstd = sA.tile([128, 2, 3], F32, name="rstd")
                if bb == 0:
                    nc.scalar.activation(rstd, mvp[:, :, :, 1], AF.Sqrt,
                                         bias=eps_sb)
                    nc.vector.reciprocal(out=rstd, in_=rstd)
                else:
                    # DVE-only Newton rsqrt: the ACT Sqrt lives in a different
                    # activation-table set than tanh/exp, and these pairs run
                    # interleaved with the attention loop -- each ACT sqrt
                    # would cost two ~1.3us table reloads.
                    xv = sA.tile([128, 2, 3], F32, name="rsq_x")
                    nc.vector.tensor_scalar_add(out=xv, in0=mvp[:, :, :, 1],
                                                scalar1=EPS)
                    yi = sA.tile([128, 2, 3], I32, name="rsq_yi")
                    nc.vector.tensor_scalar(
                        out=yi, in0=xv.bitcast(I32), scalar1=1, scalar2=None,
                        op0=ALU.logical_shift_right)
                    nc.vector.tensor_tensor(out=yi, in0=magic_sb, in1=yi,
                                            op=ALU.subtract)
                    y = yi.bitcast(F32)
                    h = sA.tile([128, 2, 3], F32, name="rsq_h")
                    for _ in range(3):
                        nc.vector.tensor_tensor(out=h, in0=y, in1=y,
                                                op=ALU.mult)
                        nc.vector.tensor_tensor(out=h, in0=h, in1=xv,
                                                op=ALU.mult)
                        nc.vector.tensor_scalar(
                            out=h, in0=h, scalar1=-0.5, scalar2=1.5,
                            op0=ALU.mult, op1=ALU.add)
                        nc.vector.tensor_tensor(out=y, in0=y, in1=h,
                                                op=ALU.mult)
                    nc.vector.tensor_copy(out=rstd, in_=y)

                for pp, (bb_, ktile_, ps_) in enumerate([held, (bb, ktile, qkv_sb)]):
                    qn = sA.tile([128, 128], BF16, name="qn")
                    nc.vector.tensor_scalar(
                        out=qn, in0=ps_[:, 0:DQK],
                        scalar1=mvp[:, pp, 0, 0:1], scalar2=rstd[:, pp, 0:1],
                        op0=ALU.subtract, op1=ALU.mult,
                    )
                    kn = sA.tile([128, 128], BF16, name="kn")
                    nc.vector.tensor_scalar(
                        out=kn, in0=ps_[:, DQK:2 * DQK],
                        scalar1=mvp[:, pp, 1, 0:1], scalar2=rstd[:, pp, 1:2],
                        op0=ALU.subtract, op1=ALU.mult,
                    )
                    nc.vector.tensor_scalar(
                        out=v_sb[bb_][:, ktile_, 0:DV], in0=ps_[:, 2 * DQK:WCOLS],
                        scalar1=mvp[:, pp, 2, 0:1], scalar2=rstd[:, pp, 2:3],
                        op0=ALU.subtract, op1=ALU.mult,
                    )
                    pending_tr.append((bb_, ktile_, qn, kn, bb_ == 1))

            def flush_tr():
                while pending_tr:
                    emit_tr(*pending_tr.pop(0))

            # batch 0's projection runs standalone; batch 1's row-tile pairs
            # are injected into batch 0's attention loop below so the PE
            # queue stays dense across the phase boundary.
            for rt in range(RT_PER_B):
                emit_rt(rt)
            flush_tr()
            # prefetch batch 1's x chunks on the (idle) scalar HWDGE queue so
            # the injected projection matmuls never block the PE FIFO on DMA
            for xc in range(RT_PER_B // RT_PER_XC, (b * RT_PER_B) // RT_PER_XC):
                load_xt(xc, engine=nc.scalar)
            inject_pairs = list(range(RT_PER_B, b * RT_PER_B, 2))

            # ---------------- stage B: attention + output projection ----------------
            # attn@v matmuls are emitted one kt-iteration behind their sim so
            # the PE never blocks on the DVE->ACT->ACT chain; the previous
            # qhalf's output projection is drip-fed into the kt loop.
            # w_out is loaded here (not with the other constants) so the head
            # DMA queue serves stage A's x/w tiles first.
            w_out_a = const.tile([128, dim], BF16)
            nc.sync.dma_start(out=w_out_a, in_=w_out[0:128, :])
            w_out_b = const.tile([64, dim], BF16)
            nc.sync.dma_start(out=w_out_b, in_=w_out[128:192, :])

            def emit_po(outUa_, outUb_, rcol_, bb_, qoff_, t, cc, tail=False):
                po = psW.tile([128, 512], F32, name="po", tag="w")
                nc.tensor.matmul(
                    po,
                    lhsT=outUa_[:, t * 128:(t + 1) * 128],
                    rhs=w_out_a[:, cc * 512:(cc + 1) * 512],
                    start=True, stop=False,
                )
                nc.tensor.matmul(
                    po,
                    lhsT=outUb_[:, t * 128:(t + 1) * 128],
                    rhs=w_out_b[:, cc * 512:(cc + 1) * 512],
                    start=False, stop=True,
                )
                po_sb = sB.tile([128, 512], BF16, name="po_sb")
                on_act = (t * CC + cc) % 2 == 1 if tail else (t * CC + cc) % 3 == 2
                if on_act:
                    nc.scalar.activation(
                        out=po_sb, in_=po, func=AF.Copy, scale=rcol_[:, t:t + 1])
                else:
                    nc.vector.tensor_scalar_mul(
                        out=po_sb, in0=po, scalar1=rcol_[:, t:t + 1])
                nc.sync.dma_start(
                    out=out[bb_, qoff_ + t * 128: qoff_ + (t + 1) * 128,
                            cc * 512:(cc + 1) * 512],
                    in_=po_sb,
                )

            import functools

            KT2 = KT // 2
            pending_po = []
            pending_evac = []
            for bb in range(b):
                for qh in range(QH):
                    qoff = qh * QHW
                    accA = [psAcc.tile([128, 512], F32, name=f"accA{qc}") for qc in range(QC)]
                    accB = [psAcc.tile([65, 512], F32, name=f"accB{qc}") for qc in range(QC)]

                    def fire_av(p, ep):
                        if AV_FP8:
                            # fp8 DoubleRow: one matmul covers key-tiles 2p, 2p+1.
                            for qc in range(QC):
                                nc.tensor.matmul(
                                    accA[qc],
                                    lhsT=v_sb[bb][:, 2 * p:2 * p + 2, 0:128],
                                    rhs=ep[:, :, qc * 512:(qc + 1) * 512],
                                    start=(p == 0), stop=(p == KT2 - 1),
                                    perf_mode=mybir.MatmulPerfMode.DoubleRow,
                                )
                                nc.tensor.matmul(
                                    accB[qc],
                                    lhsT=v_sb[bb][:, 2 * p:2 * p + 2, 128:DV + 1],
                                    rhs=ep[:, :, qc * 512:(qc + 1) * 512],
                                    start=(p == 0), stop=(p == KT2 - 1),
                                    perf_mode=mybir.MatmulPerfMode.DoubleRow,
                                )
                        else:
                            for s in range(2):
                                pkt = 2 * p + s
                                for qc in range(QC):
                                    nc.tensor.matmul(
                                        accA[qc],
                                        lhsT=v_sb[bb][:, pkt, 0:128],
                                        rhs=ep[:, s, qc * 512:(qc + 1) * 512],
                                        start=(pkt == 0), stop=(pkt == KT - 1),
                                    )
                                    nc.tensor.matmul(
                                        accB[qc],
                                        lhsT=v_sb[bb][:, pkt, 128:DV + 1],
                                        rhs=ep[:, s, qc * 512:(qc + 1) * 512],
                                        start=(pkt == 0), stop=(pkt == KT - 1),
                                    )

                    pending_av = []
                    e_pair = None
                    for kt in range(KT):
                        th_sb = sB.tile([128, QHW], F32, name="th_sb")
                        for qc in range(QC):
                            bias_sb = sBias.tile([128, 512], BF16, name="bias_sb")
                            nc.sync.dma_start(
                                out=bias_sb,
                                in_=biasT[bb, kt * 128:(kt + 1) * 128,
                                          qoff + qc * 512: qoff + (qc + 1) * 512],
                            )
                            sim_ps = psW.tile([128, 512], F32, name="sim_ps", tag="w")
                            nc.tensor.matmul(
                                sim_ps,
                                lhsT=kT_sb[bb][:, kt * 128:(kt + 1) * 128],
                                rhs=qT_sb[bb][:, qoff + qc * 512: qoff + (qc + 1) * 512],
                                start=True, stop=False,
                            )
                            # bias-add on the PE: identity matmul accumulates
                            # the bias tile into the sim PSUM bank, keeping
                            # the sim->tanh chain off the (FIFO-ordered) DVE.
                            nc.tensor.matmul(
                                sim_ps,
                                lhsT=ident,
                                rhs=bias_sb,
                                start=False, stop=True,
                            )
                            nc.scalar.activation(
                                th_sb[:, qc * 512:(qc + 1) * 512], sim_ps,
                                AF.Tanh, scale=1.0 / CLAMP)
                        if kt % 2 == 0:
                            e_pair = sE.tile([128, 2, QHW], VDT, name="e_pair")
                        nc.scalar.activation(e_pair[:, kt % 2, :], th_sb,
                                             AF.Exp, scale=CLAMP)
                        if kt % 2 == 1:
                            pending_av.append((kt // 2, e_pair))

                        if len(pending_av) >= 3:
                            fire_av(*pending_av.pop(0))

                        if kt >= 2:
                            for _ in range(2):
                                if pending_po:
                                    pending_po.pop(0)()

                        if bb == 0 and kt % 4 == 2 and inject_pairs:
                            r0 = inject_pairs.pop(0)
                            emit_rt(r0)
                            emit_rt(r0 + 1)

                    for p, ep in pending_av:
                        fire_av(p, ep)

                    def make_evac(accA_, accB_, bb_, qoff_):  # noqa: unused-now-eager
                        # Evacuate accumulators with plain casts. The
                        # attention row-sums (65th accB row) are reshaped into
                        # per-partition columns via a DRAM bounce, then one
                        # wide reciprocal gives the 1/s scale column that the
                        # output-projection copy applies per token.
                        def evac():
                            outUa = sB.tile([128, QHW], BF16, name="outUa")
                            outUb = sB.tile([64, QHW], BF16, name="outUb")
                            s_row = sB.tile([1, QHW], F32, name="s_row")
                            for qc in range(QC):
                                nc.vector.tensor_copy(
                                    out=s_row[:, qc * 512:(qc + 1) * 512],
                                    in_=accB_[qc][64:65, :],
                                )
                            s_dram = sDram.tile([1, QHW], F32, name="s_dram")
                            nc.sync.dma_start(out=s_dram, in_=s_row)
                            for qc in range(QC):
                                nc.vector.tensor_copy(
                                    out=outUa[:, qc * 512:(qc + 1) * 512],
                                    in_=accA_[qc])
                                nc.vector.tensor_copy(
                                    out=outUb[:, qc * 512:(qc + 1) * 512],
                                    in_=accB_[qc][0:64, :])
                            rcol_raw = sB.tile([128, QHW // 128], F32,
                                               name="rcol_raw")
                            nc.sync.dma_start(
                                out=rcol_raw,
                                in_=s_dram.rearrange("one (t p) -> p (one t)",
                                                     p=128),
                            )
                            rcol = sB.tile([128, QHW // 128], F32, name="rcol")
                            nc.vector.reciprocal(out=rcol, in_=rcol_raw)
                            for t in range(QHW // 128):
                                for cc in range(CC):
                                    pending_po.append(functools.partial(
                                        emit_po, outUa, outUb, rcol,
                                        bb_, qoff_, t, cc))
                        return evac

                    make_evac(accA, accB, bb, qoff)()

                if bb == 0:
                    flush_tr()

            for fn in pending_po:
                fn(tail=True)

    nc.compile()
    return nc


_NC_CACHE = {}


def _get_nc(b=B, n=N, dim=DIM):
    key = (b, n, dim)
    if key not in _NC_CACHE:
        _NC_CACHE[key] = build_nc(b, n, dim)
    return _NC_CACHE[key]


def make_in_maps(x, attn_bias, w_qkv, w_out, g_q, g_k, g_v, n_cores=N_CORES):
    """Host-side shard + preprocess. Returns per-core input maps."""
    b, n, dim = x.shape
    bf = ml_dtypes.bfloat16
    xT = np.ascontiguousarray(
        x.reshape(b * n, dim).T).astype(bf)                      # [dim, b*n]
    kv_cols = np.ascontiguousarray(
        w_qkv[:, HEADS * DQK:]).astype(np.float32)               # [dim, 320]
    in_maps = []
    for c in range(n_cores):
        h = c % HEADS
        w_q_h = w_qkv[:, h * DQK:(h + 1) * DQK]
        w_all = np.concatenate([w_q_h, kv_cols], axis=1).astype(bf)  # [dim, 448]
        biasT = np.ascontiguousarray(
            attn_bias[:, h, :, :].transpose(0, 2, 1)).astype(bf)  # [b, keys, qrows]
        w_out_h = (w_out[h * DV:(h + 1) * DV, :]
                   * g_v[:, None].astype(np.float32)).astype(bf)  # [dv, dim]
        in_maps.append({
            "xT": xT,
            "w_all": w_all,
            "biasT": biasT,
            "w_out": w_out_h,
            "gq": (g_q * SCALE).astype(np.float32).reshape(DQK, 1),
            "gk": g_k.astype(np.float32).reshape(DQK, 1),
        })
    return in_maps


def kernel(x, attn_bias, w_qkv, w_out, g_q, g_k, g_v):
    x = np.asarray(x, dtype=np.float32)
    attn_bias = np.asarray(attn_bias, dtype=np.float32)
    w_qkv = np.asarray(w_qkv, dtype=np.float32)
    w_out = np.asarray(w_out, dtype=np.float32)
    g_q = np.asarray(g_q, dtype=np.float32)
    g_k = np.asarray(g_k, dtype=np.float32)
    g_v = np.asarray(g_v, dtype=np.float32)

    b, n, dim = x.shape
    nc = _get_nc(b, n, dim)
    in_maps = make_in_maps(x, attn_bias, w_qkv, w_out, g_q, g_k, g_v)
    res = run_bass_kernel_spmd(nc, in_maps, core_ids=list(range(N_CORES)),
                               trace=bool(os.environ.get("KERNEL_TRACE")))
    _LAST_STATS["exec_time_ns"] = res.exec_time_ns
    _LAST_STATS["mean_exec_time_ns"] = res.mean_exec_time_ns
    _LAST_STATS["res"] = res
    out = np.zeros((b, n, dim), dtype=np.float32)
    for c in range(N_CORES):
        out += res.results[c]["out"].astype(np.float32)
    return out

